# revision 1
# baseline (speedup 1.0000x reference)
"""Causal self-attention on 8 TRN2 NeuronCores.

Problem: B=4, S=2048, D=1024, H=16 heads (hd=64), fp32 in/out.
  qkv = x @ w_qkv + b_qkv ; causal softmax attention ; y @ w_out + b_out

Sharding (tensor-parallel over heads x data-parallel over batch):
  core c -> batch b = c//2, head-group hg = c%2 (8 heads each).
  Each core computes qkv for its 8 heads from x[b], runs attention, and
  produces a partial output  y_local @ w_out[rows]  of shape [S, D].
  Host unshards: out[b] = partial[2b] + partial[2b+1] + b_out.

Device kernel (per core), bf16 matmul operands / fp32 PSUM accumulation:
  - x passed transposed (xT [D, S], bf16) so both projections contract D
    on partitions with no device-side transposes.
  - q,k produced directly transposed (qT/kT [64, S] per head) via
    out = w.T @ x; heads processed in pairs packed at partition offsets
    0-63 / 64-127.  v in natural layout with a ones column (v_aug) so
    the PV matmul also produces the softmax denominator.
  - scores computed transposed (S_T[k, q]) so attT = exp(S_T) is already
    in PV layout; no attention-matrix transposes.  Causal handled by
    block skipping; on diagonal tiles the scores/mask/exp/PV all operate
    only on the live column strip (additive -1e30 mask windows).
  - softmax denominator: ones column accumulates rowsum into row 64 of
    the PV psum; 1/rowsum = exp(-ln(.)) on ACT, DMA partition-shift to
    partition 0, gpsimd partition_broadcast, then the normalization
    multiply fuses into the psum->SBUF eviction of yT.  Max-subtraction
    is skipped (scores are O(1) by construction; exp exact in fp32).
"""

import os
import sys

for _p in ("/root/.axon_site/_ro/trn_rl_repo", "/opt/trn_rl_repo"):
    if os.path.isdir(_p) and _p not in sys.path:
        sys.path.append(_p)

import ml_dtypes
import numpy as np

import concourse.bass as bass  # noqa: F401
import concourse.mybir as mybir
import concourse.tile as tile
from concourse import bacc
from concourse.bass_utils import run_bass_kernel_spmd

B, S, D, H = 4, 2048, 1024, 16
HD = 64
HPC = 8          # heads per core
NPAIR = HPC // 2
KO = D // 128    # contraction chunks over D
ATT_SCALE = 1.0 / np.sqrt(HD)
NEG = -1.0e30

F32 = mybir.dt.float32
F32R = mybir.dt.float32r
BF16 = mybir.dt.bfloat16
NPBF16 = ml_dtypes.bfloat16


def build_nc(S_=S):
    KT = S_ // 128    # k tiles
    TB = S_ // 512    # token blocks for projections

    nc = bacc.Bacc(None)
    xT_d = nc.dram_tensor("xT", [D, S_], BF16, kind="ExternalInput")
    wqk_d = nc.dram_tensor("wqk", [D, NPAIR, 2, 128], BF16, kind="ExternalInput")
    bqk_d = nc.dram_tensor("bqk", [128, NPAIR, 2], F32, kind="ExternalInput")
    wv_d = nc.dram_tensor("wv", [D, HPC * HD], BF16, kind="ExternalInput")
    bv_d = nc.dram_tensor("bv", [128, HPC * HD], F32, kind="ExternalInput")
    wout_d = nc.dram_tensor("wout", [HPC * HD, D], BF16, kind="ExternalInput")
    mask_d = nc.dram_tensor("mask", [128, 896], F32, kind="ExternalInput")
    out_d = nc.dram_tensor("out", [S_, D], F32, kind="ExternalOutput")

    with tile.TileContext(nc) as tc, nc.allow_low_precision("bf16/f32r matmul operands"):
        with (
            tc.tile_pool(name="const", bufs=1) as constp,
            tc.tile_pool(name="psA", bufs=2, space="PSUM") as psA,
            tc.tile_pool(name="psS", bufs=4, space="PSUM") as psS,
            tc.tile_pool(name="psY", bufs=2, space="PSUM") as psY,
        ):
            mask_sb = constp.tile([128, 896], F32)
            nc.sync.dma_start(mask_sb[:], mask_d[:])
            bqk_sb = constp.tile([128, NPAIR, 2], F32)
            nc.sync.dma_start(bqk_sb[:], bqk_d[:])
            bv_sb = constp.tile([128, HPC * HD], F32)
            nc.sync.dma_start(bv_sb[:], bv_d[:])
            # v with ones column (col 64); col 65 is pad
            vaug = constp.tile([128, KT, HPC, 66], BF16)
            nc.gpsimd.memset(vaug[:, :, :, 64], 1.0)
            yT = constp.tile([128, NPAIR, S_], BF16)

            with tc.tile_pool(name="px", bufs=1) as px:
                xT = px.tile([128, KO, S_], BF16)
                xr = xT_d.rearrange("(ko p) t -> p ko t", p=128)
                # k-chunk split: few large contiguous descriptors (a token
                # split was measured much slower despite earlier compute start)
                for i in range(4):
                    nc.sync.dma_start(xT[:, 2 * i : 2 * i + 2, :], xr[:, 2 * i : 2 * i + 2, :])

                # ---- v projection (all heads), biased, into v_aug ----
                with tc.tile_pool(name="pwv", bufs=1) as pwv:
                    wv_sb = pwv.tile([128, KO, HPC * HD], BF16)
                    nc.sync.dma_start(wv_sb[:], wv_d.rearrange("(ko p) c -> p ko c", p=128))
                    for tt in range(KT):
                        ps = psA.tile([128, 512], F32, tag="psA")
                        for k in range(KO):
                            nc.tensor.matmul(
                                ps,
                                xT[:, k, tt * 128 : (tt + 1) * 128],
                                wv_sb[:, k, :],
                                start=(k == 0),
                                stop=(k == KO - 1),
                            )
                        nc.vector.tensor_tensor(
                            vaug[:, tt, :, 0:64],
                            ps[:].rearrange("p (h d) -> p h d", h=HPC),
                            bv_sb[:].rearrange("p (h d) -> p h d", h=HPC),
                            mybir.AluOpType.add,
                        )

                with (
                    tc.tile_pool(name="pqk", bufs=2) as pqk,
                    tc.tile_pool(name="pw", bufs=2) as pw,
                    tc.tile_pool(name="patt", bufs=4) as patt,
                    tc.tile_pool(name="pnorm", bufs=2) as pnorm,
                ):
                    for pr in range(NPAIR):
                        # ---- q/k projection for head pair, packed 64|64 ----
                        wqk_sb = pw.tile([128, KO, 2, 128], BF16, tag="wqk")
                        nc.sync.dma_start(
                            wqk_sb[:],
                            wqk_d.rearrange("(ko p) r c2 c -> p ko r c2 c", p=128)[
                                :, :, pr, :, :
                            ],
                        )
                        qT = pqk.tile([128, S_], BF16, tag="qT")
                        kT = pqk.tile([128, S_], BF16, tag="kT")
                        for cqk in range(2):
                            dst = qT if cqk == 0 else kT
                            for tb0 in range(0, TB, 2):
                                tbs = [tb0] + ([tb0 + 1] if tb0 + 1 < TB else [])
                                pst = [
                                    psA.tile([128, 512], F32, tag="psA", name=f"pj{i}")
                                    for i in range(len(tbs))
                                ]
                                for k in range(KO):
                                    for i, tb in enumerate(tbs):
                                        nc.tensor.matmul(
                                            pst[i],
                                            wqk_sb[:, k, cqk, :],
                                            xT[:, k, tb * 512 : (tb + 1) * 512],
                                            start=(k == 0),
                                            stop=(k == KO - 1),
                                        )
                                for i, tb in enumerate(tbs):
                                    nc.vector.tensor_scalar_add(
                                        dst[:, tb * 512 : (tb + 1) * 512],
                                        pst[i][:],
                                        bqk_sb[:, pr, cqk : cqk + 1],
                                    )

                        # ---- attention for both heads of the pair ----
                        for a in range(S_ // 512):
                            psy = [None, None]
                            for h01 in range(2):
                                psy[h01] = psY.tile(
                                    [65, 512], F32, tag="psY", name=f"psy{h01}"
                                )
                            nj = 4 * a + 4
                            for j in range(nj):
                                o = 128 * j - 512 * a
                                # phase-grouped so the two K=64 score matmuls
                                # sit adjacent in the PE queue and pack onto
                                # disjoint row halves of the array
                                # diagonal tiles: only columns >= o are live;
                                # compute scores/mask/exp on the live strip and
                                # zero the rest of attT
                                oo = max(o, 0)
                                W = 512 - oo
                                pss2, att2 = [], []
                                for h01 in range(2):
                                    lo, hi = h01 * 64, h01 * 64 + 64
                                    pss = psS.tile(
                                        [128, 512], F32, tag="psS", name=f"pss{h01}"
                                    )
                                    nc.tensor.matmul(
                                        pss[:, 0:W],
                                        kT[lo:hi, j * 128 : (j + 1) * 128],
                                        qT[lo:hi, a * 512 + oo : (a + 1) * 512],
                                        start=True,
                                        stop=True,
                                    )
                                    pss2.append(pss)
                                for h01 in range(2):
                                    if o >= 0:
                                        nc.vector.tensor_tensor(
                                            pss2[h01][:, 0:W],
                                            pss2[h01][:, 0:W],
                                            mask_sb[:, 384 : 896 - oo],
                                            mybir.AluOpType.add,
                                        )
                                    att = patt.tile(
                                        [128, 512], BF16, tag="att", name=f"att{h01}"
                                    )
                                    nc.scalar.activation(
                                        att[:, oo:512],
                                        pss2[h01][:, 0:W],
                                        mybir.ActivationFunctionType.Exp,
                                        scale=float(ATT_SCALE),
                                    )
                                    att2.append(att)
                                for h01 in range(2):
                                    # PV restricted to the live strip; PSUM
                                    # has_written bits keep untouched columns
                                    nc.tensor.matmul(
                                        psy[h01][:, oo:512],
                                        vaug[:, j, 2 * pr + h01, 0:65],
                                        att2[h01][:, oo:512],
                                        start=(j == 0),
                                        stop=(j == nj - 1),
                                        skip_group_check=True,
                                    )
                            # ---- normalize + write yT ----
                            for h01 in range(2):
                                # 1/rowsum = exp(-ln(rowsum)) on ACT: costs a
                                # table swap but keeps the 3.3us 1-lane DVE
                                # reciprocal off the in-order DVE (measured
                                # faster than either DVE variant)
                                rtmp = pnorm.tile([65, 512], F32, tag="rt")
                                nc.scalar.activation(
                                    rtmp[64:65, :],
                                    psy[h01][64:65, :],
                                    mybir.ActivationFunctionType.Ln,
                                )
                                nc.scalar.activation(
                                    rtmp[64:65, :],
                                    rtmp[64:65, :],
                                    mybir.ActivationFunctionType.Exp,
                                    scale=-1.0,
                                )
                                rr0 = pnorm.tile([1, 512], F32, tag="rr0")
                                nc.sync.dma_start(rr0[:], rtmp[64:65, :])
                                bc = pnorm.tile([64, 512], F32, tag="bc")
                                nc.gpsimd.partition_broadcast(bc[:], rr0[:])
                                dsts = a * 512
                                if h01 == 0:
                                    nc.vector.tensor_tensor(
                                        yT[0:64, pr, dsts : dsts + 512],
                                        psy[h01][0:64, :],
                                        bc[:],
                                        mybir.AluOpType.mult,
                                    )
                                else:
                                    stg = pnorm.tile([64, 512], BF16, tag="stg")
                                    nc.vector.tensor_tensor(
                                        stg[:],
                                        psy[h01][0:64, :],
                                        bc[:],
                                        mybir.AluOpType.mult,
                                    )
                                    nc.sync.dma_start(
                                        yT[64:128, pr, dsts : dsts + 512], stg[:]
                                    )

            # ---- output projection: partial = yT.T @ w_out ----
            with tc.tile_pool(name="pout", bufs=1) as pout, tc.tile_pool(
                name="postage", bufs=3
            ) as postage:
                wout_sb = pout.tile([128, NPAIR, D], BF16)
                nc.sync.dma_start(wout_sb[:], wout_d.rearrange("(cc p) c -> p cc c", p=128))
                for tt in range(S_ // 128):
                    for nh in range(2):
                        ps = psA.tile([128, 512], F32, tag="psA")
                        for cc in range(NPAIR):
                            nc.tensor.matmul(
                                ps,
                                yT[:, cc, tt * 128 : (tt + 1) * 128],
                                wout_sb[:, cc, nh * 512 : (nh + 1) * 512],
                                start=(cc == 0),
                                stop=(cc == NPAIR - 1),
                            )
                        ot = postage.tile([128, 512], F32, tag="ot")
                        nc.vector.tensor_copy(ot[:], ps[:])
                        nc.sync.dma_start(
                            out_d[tt * 128 : (tt + 1) * 128, nh * 512 : (nh + 1) * 512], ot[:]
                        )

    nc.finalize()
    return nc


def make_host_inputs(x, w_qkv, b_qkv, w_out, b_out, S_=S):
    """Build the 8 per-core input maps (host-side shard/pack/cast)."""
    x = np.asarray(x, dtype=np.float32)
    w_qkv = np.asarray(w_qkv, dtype=np.float32)
    b_qkv = np.asarray(b_qkv, dtype=np.float32)
    w_out = np.asarray(w_out, dtype=np.float32)

    mask = np.where(
        np.arange(896)[None, :] >= np.arange(128)[:, None] + 384, 0.0, NEG
    ).astype(np.float32)

    per_hg = {}
    for hg in range(2):
        wqk = np.empty((D, NPAIR, 2, 128), np.float32)
        bqk = np.empty((128, NPAIR, 2), np.float32)
        for p in range(NPAIR):
            h0, h1 = hg * HPC + 2 * p, hg * HPC + 2 * p + 1
            wqk[:, p, 0, 0:64] = w_qkv[:, h0 * HD : (h0 + 1) * HD]
            wqk[:, p, 0, 64:128] = w_qkv[:, h1 * HD : (h1 + 1) * HD]
            wqk[:, p, 1, 0:64] = w_qkv[:, D + h0 * HD : D + (h0 + 1) * HD]
            wqk[:, p, 1, 64:128] = w_qkv[:, D + h1 * HD : D + (h1 + 1) * HD]
            bqk[0:64, p, 0] = b_qkv[h0 * HD : (h0 + 1) * HD]
            bqk[64:128, p, 0] = b_qkv[h1 * HD : (h1 + 1) * HD]
            bqk[0:64, p, 1] = b_qkv[D + h0 * HD : D + (h0 + 1) * HD]
            bqk[64:128, p, 1] = b_qkv[D + h1 * HD : D + (h1 + 1) * HD]
        wv = w_qkv[:, 2 * D + hg * 512 : 2 * D + (hg + 1) * 512]
        bv = np.broadcast_to(
            b_qkv[2 * D + hg * 512 : 2 * D + (hg + 1) * 512], (128, 512)
        ).copy()
        wout = w_out[hg * 512 : (hg + 1) * 512, :]
        per_hg[hg] = dict(
            wqk=np.ascontiguousarray(wqk.astype(NPBF16)),
            bqk=bqk,
            wv=np.ascontiguousarray(wv.astype(NPBF16)),
            bv=bv,
            wout=np.ascontiguousarray(wout.astype(NPBF16)),
        )

    xT_by_b = [
        np.ascontiguousarray(x[b, :S_].T.astype(NPBF16)) for b in range(B)
    ]
    in_maps = []
    for c in range(8):
        b, hg = c // 2, c % 2
        m = dict(per_hg[hg])
        m["xT"] = xT_by_b[b]
        m["mask"] = mask
        in_maps.append(m)
    return in_maps


_NC_CACHE = {}


def _get_nc(S_=S):
    if S_ not in _NC_CACHE:
        _NC_CACHE[S_] = build_nc(S_)
    return _NC_CACHE[S_]


def kernel(x, w_qkv, b_qkv, w_out, b_out):
    x = np.asarray(x, dtype=np.float32)
    b_out = np.asarray(b_out, dtype=np.float32)
    in_maps = make_host_inputs(x, w_qkv, b_qkv, w_out, b_out)
    nc = _get_nc()
    res = run_bass_kernel_spmd(nc, in_maps, list(range(8))).results
    out = np.empty((B, S, D), np.float32)
    for b in range(B):
        out[b] = res[2 * b]["out"] + res[2 * b + 1]["out"] + b_out[None, :]
    return out



# revision 3
# speedup vs baseline: 1.2405x; 1.2405x over previous
"""Causal self-attention on 8 TRN2 NeuronCores.

Problem: B=4, S=2048, D=1024, H=16 heads (hd=64), fp32 in/out.
  qkv = x @ w_qkv + b_qkv ; causal softmax attention ; y @ w_out + b_out

Sharding (tensor-parallel over heads x data-parallel over batch):
  core c -> batch b = c//2, head-group hg = c%2 (8 heads each).
  Each core computes qkv for its 8 heads from x[b], runs attention, and
  produces a partial output  y_local @ w_out[rows]  of shape [S, D].
  Host unshards: out[b] = partial[2b] + partial[2b+1] + b_out.

Device kernel (per core), bf16 matmul operands / fp32 PSUM accumulation:
  - x passed transposed (xT [D, S], bf16) so both projections contract D
    on partitions with no device-side transposes.  Input DMAs are split
    across the sync and gpsimd trigger queues with the q/k weights ahead
    of the bulk xT transfer so the first projection matmuls start early
    instead of queueing behind 4MB of activations.
  - q,k produced directly transposed (qT/kT [64, S] per head) via
    out = w.T @ x; heads processed in pairs packed at partition offsets
    0-63 / 64-127 (the two K=64 score matmuls then run concurrently on
    disjoint row halves of the PE array via auto tile_position).
  - scores computed transposed (S_T[k, q]) so attT = exp(S_T) is already
    in PV layout.  Both heads of a pair share one [128,1024] PSUM tile
    (head0 | head1, 2 banks) so the softmax exp is a single ACTIVATE
    over both heads' scores -- half the ACT instruction overhead of the
    kernel's hot loop, which is ACT-bound.  Causal handled by block
    skipping; on diagonal tiles the scores/mask/PV operate only on the
    live column strip (additive -1e30 mask windows); the exp covers the
    dead gap between the heads' strips (garbage exp'd, never consumed).
  - v in natural layout with a ones column (v_aug) so the PV matmul also
    produces the softmax denominator.  1/rowsum = exp(-ln(.)) on ACT --
    Exp and Ln are forced into the single combined
    natural_log_exp_and_others table set (see _patch_act_tables) so the
    scalar engine never swaps spline tables (the baseline paid 33 table
    loads / 42us); the normalization multiply fuses into the psum->SBUF
    eviction of yT.  Max-subtraction is skipped (scores are O(1) by
    construction; exp exact in fp32).
  - The attention inner loop is ACT(exp)-bound, so all other PE work is
    software-pipelined into it as emission-order "filler": the v
    projection feeds pair 0's loop just ahead of its PV consumers, the
    q/k projection for pair p+1 fills pair p's loop, and the output
    projection for finished token blocks fills pair 3's loop.  Scores
    for key tile j+1 are emitted before PV of tile j so the in-order PE
    queue never stalls on the exp feeding the next PV.  This keeps the
    PE busy enough that the HAM clock gate stays at full rate (the
    baseline ran ~45% of the kernel at half PE clock).
"""

import os
import sys

for _p in ("/root/.axon_site/_ro/trn_rl_repo", "/opt/trn_rl_repo"):
    if os.path.isdir(_p) and _p not in sys.path:
        sys.path.append(_p)

import ml_dtypes
import numpy as np

import concourse.bass as bass  # noqa: F401
import concourse.mybir as mybir
import concourse.tile as tile
from concourse import bacc
from concourse.bass_utils import run_bass_kernel_spmd

B, S, D, H = 4, 2048, 1024, 16
HD = 64
HPC = 8          # heads per core
NPAIR = HPC // 2
KO = D // 128    # contraction chunks over D
ATT_SCALE = 1.0 / np.sqrt(HD)
NEG = -1.0e30

F32 = mybir.dt.float32
F32R = mybir.dt.float32r
BF16 = mybir.dt.bfloat16
NPBF16 = ml_dtypes.bfloat16


def _patch_act_tables(arch: str):
    """Make natural_log_exp_and_others the only provider of Exp and Ln.

    The table-load insertion pass picks, per activation, a table set
    containing its function; with Exp and Ln drawn from different sets
    it emits an ACT_TABLE_LOAD (~1.3us + drain) at every alternation.
    Both functions live together in natural_log_exp_and_others, so
    removing them from every other set (entries and their positions kept,
    so act_func_set_id still indexes act_info.json correctly) forces the
    single combined set: one load for the whole kernel.
    """
    from concourse.hw_specs import get_activation_tables

    tables = get_activation_tables(arch)  # functools.cache'd: mutate in place
    exp_t = mybir.ActivationFunctionType.Exp
    ln_t = mybir.ActivationFunctionType.Ln
    combined = None
    for name, fns in tables.items():
        if exp_t in fns and ln_t in fns:
            combined = name
            break
    if combined is None:
        return
    for name, fns in tables.items():
        if name != combined:
            fns.discard(exp_t)
            fns.discard(ln_t)


def build_nc(S_=S):
    KT = S_ // 128    # key tiles
    TB = S_ // 512    # token blocks for projections
    NA = S_ // 512    # query blocks

    nc = bacc.Bacc(None)
    _patch_act_tables(nc.m.arch)
    xT_d = nc.dram_tensor("xT", [D, S_], BF16, kind="ExternalInput")
    wqk_d = nc.dram_tensor("wqk", [D, NPAIR, 2, 128], BF16, kind="ExternalInput")
    bqk_d = nc.dram_tensor("bqk", [128, NPAIR, 2], F32, kind="ExternalInput")
    wv_d = nc.dram_tensor("wv", [D, HPC * HD], BF16, kind="ExternalInput")
    bv_d = nc.dram_tensor("bv", [128, HPC * HD], F32, kind="ExternalInput")
    wout_d = nc.dram_tensor("wout", [HPC * HD, D], BF16, kind="ExternalInput")
    mask_d = nc.dram_tensor("mask", [128, 896], F32, kind="ExternalInput")
    out_d = nc.dram_tensor("out", [S_, D], F32, kind="ExternalOutput")

    with tile.TileContext(nc) as tc, nc.allow_low_precision("bf16/f32r matmul operands"):
        with (
            tc.tile_pool(name="const", bufs=1) as constp,
            tc.tile_pool(name="pw", bufs=2) as pw,
            tc.tile_pool(name="pqk", bufs=2) as pqk,
            tc.tile_pool(name="patt", bufs=4) as patt,
            tc.tile_pool(name="pnorm", bufs=2) as pnorm,
            tc.tile_pool(name="postage", bufs=3) as postage,
            tc.tile_pool(name="psA", bufs=2, space="PSUM") as psA,
            tc.tile_pool(name="psS", bufs=2, space="PSUM") as psS,
            tc.tile_pool(name="psY", bufs=2, space="PSUM") as psY,
        ):
            # ---- input DMAs: q/k weights ahead of bulk xT, split over
            # the sync and gpsimd trigger queues so they land in parallel
            bqk_sb = constp.tile([128, NPAIR, 2], F32)
            nc.sync.dma_start(bqk_sb[:], bqk_d[:])
            wqk_r = wqk_d.rearrange("(ko p) r c2 c -> p ko r c2 c", p=128)
            wqk_sb0 = pw.tile([128, KO, 2, 128], BF16, tag="wqk", name="wqk0")
            nc.sync.dma_start(wqk_sb0[:], wqk_r[:, :, 0, :, :])
            xT = constp.tile([128, KO, S_], BF16)
            xr = xT_d.rearrange("(ko p) t -> p ko t", p=128)
            for i in range(2):
                nc.sync.dma_start(xT[:, 2 * i : 2 * i + 2, :], xr[:, 2 * i : 2 * i + 2, :])
                nc.gpsimd.dma_start(
                    xT[:, 4 + 2 * i : 6 + 2 * i, :], xr[:, 4 + 2 * i : 6 + 2 * i, :]
                )
            bv_sb = constp.tile([128, HPC * HD], F32)
            nc.sync.dma_start(bv_sb[:], bv_d[:])
            wv_sb = constp.tile([128, KO, HPC * HD], BF16)
            nc.sync.dma_start(wv_sb[:], wv_d.rearrange("(ko p) c -> p ko c", p=128))
            wout_sb = constp.tile([128, NPAIR, D], BF16)
            nc.gpsimd.dma_start(wout_sb[:], wout_d.rearrange("(cc p) c -> p cc c", p=128))
            mask_sb = constp.tile([128, 896], F32)
            nc.gpsimd.dma_start(mask_sb[:], mask_d[:])

            # v with ones column (col 64); col 65 is pad
            vaug = constp.tile([128, KT, HPC, 66], BF16)
            nc.gpsimd.memset(vaug[:, :, :, 64], 1.0)
            yT = constp.tile([128, NPAIR, S_], BF16)

            wqk_tiles = {0: wqk_sb0}
            qkT = {}

            # ---- emission units (PE "filler" work) ----
            def qk_chunk(pr, cqk, tb0):
                # one double-token-block of the q or k projection: the two
                # psum tiles share each k-chunk's loaded weights
                wqk_sb = wqk_tiles[pr]
                dst = qkT[pr][cqk]
                tbs = [tb0, tb0 + 1]
                pst = [
                    psA.tile([128, 512], F32, tag="psA", name=f"pj{i}")
                    for i in range(len(tbs))
                ]
                for k in range(KO):
                    for i, tb in enumerate(tbs):
                        nc.tensor.matmul(
                            pst[i],
                            wqk_sb[:, k, cqk, :],
                            xT[:, k, tb * 512 : (tb + 1) * 512],
                            start=(k == 0),
                            stop=(k == KO - 1),
                        )
                for i, tb in enumerate(tbs):
                    nc.vector.tensor_scalar_add(
                        dst[:, tb * 512 : (tb + 1) * 512],
                        pst[i][:],
                        bqk_sb[:, pr, cqk : cqk + 1],
                    )

            def v_tile(tt):
                # v projection (all heads) for key tile tt, biased, into v_aug
                ps = psA.tile([128, 512], F32, tag="psA", name="pv")
                for k in range(KO):
                    nc.tensor.matmul(
                        ps,
                        xT[:, k, tt * 128 : (tt + 1) * 128],
                        wv_sb[:, k, :],
                        start=(k == 0),
                        stop=(k == KO - 1),
                    )
                nc.vector.tensor_tensor(
                    vaug[:, tt, :, 0:64],
                    ps[:].rearrange("p (h d) -> p h d", h=HPC),
                    bv_sb[:].rearrange("p (h d) -> p h d", h=HPC),
                    mybir.AluOpType.add,
                )

            def out_unit(tt, nh):
                # output projection for one (token tile, D half)
                ps = psA.tile([128, 512], F32, tag="psA", name="po")
                for cc in range(NPAIR):
                    nc.tensor.matmul(
                        ps,
                        yT[:, cc, tt * 128 : (tt + 1) * 128],
                        wout_sb[:, cc, nh * 512 : (nh + 1) * 512],
                        start=(cc == 0),
                        stop=(cc == NPAIR - 1),
                    )
                ot = postage.tile([128, 512], F32, tag="ot")
                nc.vector.tensor_copy(ot[:], ps[:])
                nc.sync.dma_start(
                    out_d[tt * 128 : (tt + 1) * 128, nh * 512 : (nh + 1) * 512], ot[:]
                )

            # ---- attention emission: one group = one key tile j, both
            # heads side by side in a single 2-bank PSUM tile ----
            def emit_group(pr, a, j, gname):
                qT, kT = qkT[pr]
                o = 128 * j - 512 * a
                oo = max(o, 0)
                pss = psS.tile([128, 1024], F32, tag="psS", name=f"pss{gname}")
                # the two K=64 score matmuls sit adjacent in the PE queue
                # and pack onto disjoint row halves of the array
                for h01 in range(2):
                    lo, hi = h01 * 64, h01 * 64 + 64
                    nc.tensor.matmul(
                        pss[:, h01 * 512 + oo : (h01 + 1) * 512],
                        kT[lo:hi, j * 128 : (j + 1) * 128],
                        qT[lo:hi, a * 512 + oo : (a + 1) * 512],
                        start=True,
                        stop=True,
                    )
                if o >= 0:
                    for h01 in range(2):
                        nc.vector.tensor_tensor(
                            pss[:, h01 * 512 + oo : (h01 + 1) * 512],
                            pss[:, h01 * 512 + oo : (h01 + 1) * 512],
                            mask_sb[:, 384 : 896 - oo],
                            mybir.AluOpType.add,
                        )
                att = patt.tile([128, 1024], BF16, tag="att", name=f"att{gname}")
                # one exp covers both heads (the dead [512:512+oo] gap on
                # diagonal tiles exps garbage that PV never reads)
                nc.scalar.activation(
                    att[:, oo:1024],
                    pss[:, oo:1024],
                    mybir.ActivationFunctionType.Exp,
                    scale=float(ATT_SCALE),
                )
                return att, oo

            def emit_pv(psy, att, oo, pr, a, j, nj):
                for h01 in range(2):
                    # PV restricted to the live strip; PSUM has_written
                    # bits keep untouched columns
                    nc.tensor.matmul(
                        psy[h01][:, oo:512],
                        vaug[:, j, 2 * pr + h01, 0:65],
                        att[:, h01 * 512 + oo : h01 * 512 + 512],
                        start=(j == 0),
                        stop=(j == nj - 1),
                        skip_group_check=True,
                    )

            def emit_normalize(psy, pr, a):
                dsts = a * 512
                for h01 in range(2):
                    # 1/rowsum = exp(-ln(rowsum)) on ACT (swap-free with
                    # the combined ln+exp table set); the 3.3us 1-lane
                    # DVE reciprocal on the in-order DVE measured slower
                    rtmp = pnorm.tile([65, 512], F32, tag="rt")
                    nc.scalar.activation(
                        rtmp[64:65, :],
                        psy[h01][64:65, :],
                        mybir.ActivationFunctionType.Ln,
                    )
                    nc.scalar.activation(
                        rtmp[64:65, :],
                        rtmp[64:65, :],
                        mybir.ActivationFunctionType.Exp,
                        scale=-1.0,
                    )
                    rr0 = pnorm.tile([1, 512], F32, tag="rr0")
                    nc.sync.dma_start(rr0[:], rtmp[64:65, :])
                    bc = pnorm.tile([64, 512], F32, tag="bc")
                    nc.gpsimd.partition_broadcast(bc[:], rr0[:])
                    if h01 == 0:
                        nc.vector.tensor_tensor(
                            yT[0:64, pr, dsts : dsts + 512],
                            psy[h01][0:64, :],
                            bc[:],
                            mybir.AluOpType.mult,
                        )
                    else:
                        stg = pnorm.tile([64, 512], BF16, tag="stg")
                        nc.vector.tensor_tensor(
                            stg[:],
                            psy[h01][0:64, :],
                            bc[:],
                            mybir.AluOpType.mult,
                        )
                        nc.sync.dma_start(
                            yT[64:128, pr, dsts : dsts + 512], stg[:]
                        )

            # ---- q/k projection for pair 0 runs up front (PE warm-up
            # under the tail of the xT DMA); later pairs are filler ----
            qkT[0] = (
                pqk.tile([128, S_], BF16, tag="qT", name="qT0"),
                pqk.tile([128, S_], BF16, tag="kT", name="kT0"),
            )
            for cqk in range(2):
                for tb0 in range(0, TB, 2):
                    qk_chunk(0, cqk, tb0)

            for pr in range(NPAIR):
                # prefetch next pair's q/k weights and output tiles
                if pr + 1 < NPAIR:
                    wqk_n = pw.tile(
                        [128, KO, 2, 128], BF16, tag="wqk", name=f"wqk{pr + 1}"
                    )
                    nc.sync.dma_start(wqk_n[:], wqk_r[:, :, pr + 1, :, :])
                    wqk_tiles[pr + 1] = wqk_n
                    qkT[pr + 1] = (
                        pqk.tile([128, S_], BF16, tag="qT", name=f"qT{pr + 1}"),
                        pqk.tile([128, S_], BF16, tag="kT", name=f"kT{pr + 1}"),
                    )

                # filler units interleaved into this pair's j-loop.
                # pair 0: the v-projection tiles (constrained to land
                # ahead of their PV consumers) then pair 1's q/k; pairs
                # 1-2: next pair's q/k; pair 3: output-projection units,
                # appended as their token blocks finish.
                fillers = []
                if pr == 0:
                    fillers += [("v", tt) for tt in range(KT)]
                if pr + 1 < NPAIR:
                    fillers += [
                        ("qk", pr + 1, cqk, tb0)
                        for cqk in range(2)
                        for tb0 in range(0, TB, 2)
                    ]
                f_idx = 0

                def emit_filler():
                    nonlocal f_idx
                    if f_idx >= len(fillers):
                        return False
                    f = fillers[f_idx]
                    f_idx += 1
                    if f[0] == "v":
                        v_tile(f[1])
                    elif f[0] == "qk":
                        qk_chunk(f[1], f[2], f[3])
                    else:
                        out_unit(f[1], f[2])
                    return True

                groups = [
                    (a, j, 4 * a + 4) for a in range(NA) for j in range(4 * a + 4)
                ]
                n_groups = len(groups)

                prev = None
                psy_cur = None
                for gi, (a, j, nj) in enumerate(groups):
                    if pr == 0:
                        # v tile j must be emitted before PV reads it
                        # (PV of this group is emitted next iteration)
                        while (
                            f_idx < len(fillers)
                            and fillers[f_idx][0] == "v"
                            and fillers[f_idx][1] <= j
                        ):
                            emit_filler()
                    if j == 0:
                        psy_new = [
                            psY.tile([65, 512], F32, tag="psY", name=f"psy{h}_{a}")
                            for h in range(2)
                        ]
                    att, oo = emit_group(pr, a, j, f"{pr}_{gi}")
                    # pace remaining fillers across the pair's groups
                    while f_idx < len(fillers) and f_idx * n_groups <= gi * len(fillers):
                        if not emit_filler():
                            break
                    if prev is not None:
                        p_psy, p_att, p_oo, p_a, p_j, p_nj = prev
                        emit_pv(p_psy, p_att, p_oo, pr, p_a, p_j, p_nj)
                        if p_j + 1 >= p_nj:
                            emit_normalize(p_psy, pr, p_a)
                            if pr == NPAIR - 1:
                                fillers.extend(
                                    ("out", tt, nh)
                                    for tt in range(4 * p_a, 4 * p_a + 4)
                                    for nh in range(2)
                                )
                    if j == 0:
                        psy_cur = psy_new
                    prev = (psy_cur, att, oo, a, j, nj)
                # drain the last group and any remaining fillers
                p_psy, p_att, p_oo, p_a, p_j, p_nj = prev
                emit_pv(p_psy, p_att, p_oo, pr, p_a, p_j, p_nj)
                emit_normalize(p_psy, pr, p_a)
                if pr == NPAIR - 1:
                    fillers.extend(
                        ("out", tt, nh)
                        for tt in range(4 * p_a, 4 * p_a + 4)
                        for nh in range(2)
                    )
                while emit_filler():
                    pass

    nc.finalize()
    return nc


def make_host_inputs(x, w_qkv, b_qkv, w_out, b_out, S_=S):
    """Build the 8 per-core input maps (host-side shard/pack/cast)."""
    x = np.asarray(x, dtype=np.float32)
    w_qkv = np.asarray(w_qkv, dtype=np.float32)
    b_qkv = np.asarray(b_qkv, dtype=np.float32)
    w_out = np.asarray(w_out, dtype=np.float32)

    mask = np.where(
        np.arange(896)[None, :] >= np.arange(128)[:, None] + 384, 0.0, NEG
    ).astype(np.float32)

    per_hg = {}
    for hg in range(2):
        wqk = np.empty((D, NPAIR, 2, 128), np.float32)
        bqk = np.empty((128, NPAIR, 2), np.float32)
        for p in range(NPAIR):
            h0, h1 = hg * HPC + 2 * p, hg * HPC + 2 * p + 1
            wqk[:, p, 0, 0:64] = w_qkv[:, h0 * HD : (h0 + 1) * HD]
            wqk[:, p, 0, 64:128] = w_qkv[:, h1 * HD : (h1 + 1) * HD]
            wqk[:, p, 1, 0:64] = w_qkv[:, D + h0 * HD : D + (h0 + 1) * HD]
            wqk[:, p, 1, 64:128] = w_qkv[:, D + h1 * HD : D + (h1 + 1) * HD]
            bqk[0:64, p, 0] = b_qkv[h0 * HD : (h0 + 1) * HD]
            bqk[64:128, p, 0] = b_qkv[h1 * HD : (h1 + 1) * HD]
            bqk[0:64, p, 1] = b_qkv[D + h0 * HD : D + (h0 + 1) * HD]
            bqk[64:128, p, 1] = b_qkv[D + h1 * HD : D + (h1 + 1) * HD]
        wv = w_qkv[:, 2 * D + hg * 512 : 2 * D + (hg + 1) * 512]
        bv = np.broadcast_to(
            b_qkv[2 * D + hg * 512 : 2 * D + (hg + 1) * 512], (128, 512)
        ).copy()
        wout = w_out[hg * 512 : (hg + 1) * 512, :]
        per_hg[hg] = dict(
            wqk=np.ascontiguousarray(wqk.astype(NPBF16)),
            bqk=bqk,
            wv=np.ascontiguousarray(wv.astype(NPBF16)),
            bv=bv,
            wout=np.ascontiguousarray(wout.astype(NPBF16)),
        )

    xT_by_b = [
        np.ascontiguousarray(x[b, :S_].T.astype(NPBF16)) for b in range(B)
    ]
    in_maps = []
    for c in range(8):
        b, hg = c // 2, c % 2
        m = dict(per_hg[hg])
        m["xT"] = xT_by_b[b]
        m["mask"] = mask
        in_maps.append(m)
    return in_maps


_NC_CACHE = {}


def _get_nc(S_=S):
    if S_ not in _NC_CACHE:
        _NC_CACHE[S_] = build_nc(S_)
    return _NC_CACHE[S_]


def kernel(x, w_qkv, b_qkv, w_out, b_out):
    x = np.asarray(x, dtype=np.float32)
    b_out = np.asarray(b_out, dtype=np.float32)
    in_maps = make_host_inputs(x, w_qkv, b_qkv, w_out, b_out)
    nc = _get_nc()
    res = run_bass_kernel_spmd(nc, in_maps, list(range(8))).results
    out = np.empty((B, S, D), np.float32)
    for b in range(B):
        out[b] = res[2 * b]["out"] + res[2 * b + 1]["out"] + b_out[None, :]
    return out


# revision 10
# speedup vs baseline: 1.2476x; 1.0057x over previous
"""Causal self-attention on 8 TRN2 NeuronCores.

Problem: B=4, S=2048, D=1024, H=16 heads (hd=64), fp32 in/out.
  qkv = x @ w_qkv + b_qkv ; causal softmax attention ; y @ w_out + b_out

Sharding (tensor-parallel over heads x data-parallel over batch):
  core c -> batch b = c//2, head-group hg = c%2 (8 heads each).
  Each core computes qkv for its 8 heads from x[b], runs attention, and
  produces a partial output  y_local @ w_out[rows]  of shape [S, D].
  Host unshards: out[b] = partial[2b] + partial[2b+1] + b_out.

Device kernel (per core), bf16 matmul operands / fp32 PSUM accumulation:
  - x passed transposed (xT [D, S], bf16) so both projections contract D
    on partitions with no device-side transposes.  Input DMAs are split
    across the sync and gpsimd trigger queues with the q/k weights ahead
    of the bulk xT transfer so the first projection matmuls start early
    instead of queueing behind 4MB of activations.
  - q,k produced directly transposed (qT/kT [64, S] per head) via
    out = w.T @ x; heads processed in pairs packed at partition offsets
    0-63 / 64-127 (the two K=64 score matmuls then run concurrently on
    disjoint row halves of the PE array via auto tile_position).
  - scores computed transposed (S_T[k, q]) so attT = exp(S_T) is already
    in PV layout.  Both heads of a pair share one [128,1024] PSUM tile
    (head0 | head1, 2 banks) so the softmax exp is a single ACTIVATE
    over both heads' scores -- half the ACT instruction overhead of the
    kernel's hot loop, which is ACT-bound.  Causal handled by block
    skipping; on diagonal tiles the scores/mask/PV operate only on the
    live column strip (additive -1e30 mask windows); the exp covers the
    dead gap between the heads' strips (garbage exp'd, never consumed).
  - v in natural layout with a ones column (v_aug) so the PV matmul also
    produces the softmax denominator.  1/rowsum = exp(-ln(.)) on ACT --
    Exp and Ln are forced into the single combined
    natural_log_exp_and_others table set (see _patch_act_tables) so the
    scalar engine never swaps spline tables (the baseline paid 33 table
    loads / 42us); the normalization multiply fuses into the psum->SBUF
    eviction of yT.  Max-subtraction is skipped (scores are O(1) by
    construction; exp exact in fp32).
  - The attention inner loop is ACT(exp)-bound, so all other PE work is
    software-pipelined into it as emission-order "filler": the v
    projection feeds pair 0's loop just ahead of its PV consumers, the
    q/k projection for pair p+1 fills pair p's loop, and the output
    projection for finished token blocks fills pair 3's loop.  Scores
    for key tile j+1 are emitted before PV of tile j so the in-order PE
    queue never stalls on the exp feeding the next PV.  This keeps the
    PE busy enough that the HAM clock gate stays at full rate (the
    baseline ran ~45% of the kernel at half PE clock).
"""

import os
import sys

for _p in ("/root/.axon_site/_ro/trn_rl_repo", "/opt/trn_rl_repo"):
    if os.path.isdir(_p) and _p not in sys.path:
        sys.path.append(_p)

import ml_dtypes
import numpy as np

import concourse.bass as bass  # noqa: F401
import concourse.mybir as mybir
import concourse.tile as tile
from concourse import bacc
from concourse.bass_utils import run_bass_kernel_spmd

B, S, D, H = 4, 2048, 1024, 16
HD = 64
HPC = 8          # heads per core
NPAIR = HPC // 2
KO = D // 128    # contraction chunks over D
ATT_SCALE = 1.0 / np.sqrt(HD)
NEG = -1.0e30

F32 = mybir.dt.float32
F32R = mybir.dt.float32r
BF16 = mybir.dt.bfloat16
NPBF16 = ml_dtypes.bfloat16


def _patch_act_tables(arch: str):
    """Make natural_log_exp_and_others the only provider of Exp and Ln.

    The table-load insertion pass picks, per activation, a table set
    containing its function; with Exp and Ln drawn from different sets
    it emits an ACT_TABLE_LOAD (~1.3us + drain) at every alternation.
    Both functions live together in natural_log_exp_and_others, so
    removing them from every other set (entries and their positions kept,
    so act_func_set_id still indexes act_info.json correctly) forces the
    single combined set: one load for the whole kernel.
    """
    from concourse.hw_specs import get_activation_tables

    tables = get_activation_tables(arch)  # functools.cache'd: mutate in place
    exp_t = mybir.ActivationFunctionType.Exp
    ln_t = mybir.ActivationFunctionType.Ln
    combined = None
    for name, fns in tables.items():
        if exp_t in fns and ln_t in fns:
            combined = name
            break
    if combined is None:
        return
    for name, fns in tables.items():
        if name != combined:
            fns.discard(exp_t)
            fns.discard(ln_t)


def build_nc(S_=S):
    KT = S_ // 128    # key tiles
    TB = S_ // 512    # token blocks for projections
    NA = S_ // 512    # query blocks

    nc = bacc.Bacc(None)
    _patch_act_tables(nc.m.arch)
    xT_d = nc.dram_tensor("xT", [D, S_], BF16, kind="ExternalInput")
    wqk_d = nc.dram_tensor("wqk", [D, NPAIR, 2, 128], BF16, kind="ExternalInput")
    bqk_d = nc.dram_tensor("bqk", [128, NPAIR, 2], F32, kind="ExternalInput")
    wv_d = nc.dram_tensor("wv", [D, HPC * HD], BF16, kind="ExternalInput")
    bv_d = nc.dram_tensor("bv", [128, HPC * HD], F32, kind="ExternalInput")
    wout_d = nc.dram_tensor("wout", [HPC * HD, D], BF16, kind="ExternalInput")
    # mask2[:, 0:896] is the base causal window; [896:1408] repeats
    # [384:896] so one DVE add covers both heads of a merged score tile
    mask_d = nc.dram_tensor("mask", [128, 1408], F32, kind="ExternalInput")
    out_d = nc.dram_tensor("out", [S_, D], F32, kind="ExternalOutput")

    with tile.TileContext(nc) as tc, nc.allow_low_precision("bf16/f32r matmul operands"):
        with (
            tc.tile_pool(name="const", bufs=1) as constp,
            tc.tile_pool(name="pw", bufs=2) as pw,
            tc.tile_pool(name="pqk", bufs=2) as pqk,
            tc.tile_pool(name="patt", bufs=4) as patt,
            tc.tile_pool(name="pnorm", bufs=2) as pnorm,
            tc.tile_pool(name="postage", bufs=3) as postage,
            tc.tile_pool(name="psA", bufs=2, space="PSUM") as psA,
            tc.tile_pool(name="psS", bufs=2, space="PSUM") as psS,
            tc.tile_pool(name="psY", bufs=2, space="PSUM") as psY,
        ):
            # ---- input DMAs: q/k weights ahead of bulk xT, the xT
            # chunks split over four trigger queues (sync/vector/scalar/
            # gpsimd) so descriptor generation and transfers overlap ----
            bqk_sb = constp.tile([128, NPAIR, 2], F32)
            nc.sync.dma_start(bqk_sb[:], bqk_d[:])
            wqk_r = wqk_d.rearrange("(ko p) r c2 c -> p ko r c2 c", p=128)
            wqk_sb0 = pw.tile([128, KO, 2, 128], BF16, tag="wqk", name="wqk0")
            nc.sync.dma_start(wqk_sb0[:], wqk_r[:, :, 0, :, :])
            xT = constp.tile([128, KO, S_], BF16)
            xr = xT_d.rearrange("(ko p) t -> p ko t", p=128)
            nc.sync.dma_start(xT[:, 0:2, :], xr[:, 0:2, :])
            nc.scalar.dma_start(xT[:, 2:4, :], xr[:, 2:4, :])
            nc.scalar.dma_start(xT[:, 4:6, :], xr[:, 4:6, :])
            nc.gpsimd.dma_start(xT[:, 6:8, :], xr[:, 6:8, :])
            bv_sb = constp.tile([128, HPC * HD], F32)
            nc.sync.dma_start(bv_sb[:], bv_d[:])
            wv_sb = constp.tile([128, KO, HPC * HD], BF16)
            nc.sync.dma_start(wv_sb[:], wv_d.rearrange("(ko p) c -> p ko c", p=128))
            wout_sb = constp.tile([128, NPAIR, D], BF16)
            nc.scalar.dma_start(wout_sb[:], wout_d.rearrange("(cc p) c -> p cc c", p=128))
            mask_sb = constp.tile([128, 1408], F32)
            nc.gpsimd.dma_start(mask_sb[:], mask_d[:])

            # v with ones column (col 64); col 65 is pad
            vaug = constp.tile([128, KT, HPC, 66], BF16)
            nc.gpsimd.memset(vaug[:, :, :, 64], 1.0)
            yT = constp.tile([128, NPAIR, S_], BF16)

            wqk_tiles = {0: wqk_sb0}
            qkT = {}

            # ---- emission units (PE "filler" work) ----
            def qk_chunk(pr, cqk, tb0):
                # one double-token-block of the q or k projection: the two
                # psum tiles share each k-chunk's loaded weights
                wqk_sb = wqk_tiles[pr]
                dst = qkT[pr][cqk]
                tbs = [tb0, tb0 + 1]
                pst = [
                    psA.tile([128, 512], F32, tag="psA", name=f"pj{i}")
                    for i in range(len(tbs))
                ]
                for k in range(KO):
                    for i, tb in enumerate(tbs):
                        nc.tensor.matmul(
                            pst[i],
                            wqk_sb[:, k, cqk, :],
                            xT[:, k, tb * 512 : (tb + 1) * 512],
                            start=(k == 0),
                            stop=(k == KO - 1),
                        )
                for i, tb in enumerate(tbs):
                    nc.vector.tensor_scalar_add(
                        dst[:, tb * 512 : (tb + 1) * 512],
                        pst[i][:],
                        bqk_sb[:, pr, cqk : cqk + 1],
                    )

            def v_tile(tt):
                # v projection (all heads) for key tile tt, biased, into v_aug
                ps = psA.tile([128, 512], F32, tag="psA", name="pv")
                for k in range(KO):
                    nc.tensor.matmul(
                        ps,
                        xT[:, k, tt * 128 : (tt + 1) * 128],
                        wv_sb[:, k, :],
                        start=(k == 0),
                        stop=(k == KO - 1),
                    )
                nc.vector.tensor_tensor(
                    vaug[:, tt, :, 0:64],
                    ps[:].rearrange("p (h d) -> p h d", h=HPC),
                    bv_sb[:].rearrange("p (h d) -> p h d", h=HPC),
                    mybir.AluOpType.add,
                )

            def out_unit(tt, nh):
                # output projection for one (token tile, D half)
                ps = psA.tile([128, 512], F32, tag="psA", name="po")
                for cc in range(NPAIR):
                    nc.tensor.matmul(
                        ps,
                        yT[:, cc, tt * 128 : (tt + 1) * 128],
                        wout_sb[:, cc, nh * 512 : (nh + 1) * 512],
                        start=(cc == 0),
                        stop=(cc == NPAIR - 1),
                    )
                ot = postage.tile([128, 512], F32, tag="ot")
                nc.vector.tensor_copy(ot[:], ps[:])
                nc.sync.dma_start(
                    out_d[tt * 128 : (tt + 1) * 128, nh * 512 : (nh + 1) * 512], ot[:]
                )

            # ---- attention emission: one group = one key tile j, both
            # heads side by side in a single 2-bank PSUM tile ----
            def emit_group(pr, a, j, gname):
                qT, kT = qkT[pr]
                o = 128 * j - 512 * a
                oo = max(o, 0)
                pss = psS.tile([128, 1024], F32, tag="psS", name=f"pss{gname}")
                # the two K=64 score matmuls sit adjacent in the PE queue
                # and pack onto disjoint row halves of the array
                for h01 in range(2):
                    lo, hi = h01 * 64, h01 * 64 + 64
                    nc.tensor.matmul(
                        pss[:, h01 * 512 + oo : (h01 + 1) * 512],
                        kT[lo:hi, j * 128 : (j + 1) * 128],
                        qT[lo:hi, a * 512 + oo : (a + 1) * 512],
                        start=True,
                        stop=True,
                    )
                if o >= 0:
                    # one add masks both heads: mask2's [896:1408] region
                    # repeats the window so src column 384+x-oo is correct
                    # for both live strips (the dead gap gets garbage)
                    nc.vector.tensor_tensor(
                        pss[:, oo:1024],
                        pss[:, oo:1024],
                        mask_sb[:, 384 : 1408 - oo],
                        mybir.AluOpType.add,
                    )
                att = patt.tile([128, 1024], BF16, tag="att", name=f"att{gname}")
                # one exp covers both heads (the dead [512:512+oo] gap on
                # diagonal tiles exps garbage that PV never reads)
                nc.scalar.activation(
                    att[:, oo:1024],
                    pss[:, oo:1024],
                    mybir.ActivationFunctionType.Exp,
                    scale=float(ATT_SCALE),
                )
                return att, oo

            def emit_pv(psy, att, oo, pr, a, j, nj):
                for h01 in range(2):
                    # PV restricted to the live strip; PSUM has_written
                    # bits keep untouched columns
                    nc.tensor.matmul(
                        psy[h01][:, oo:512],
                        vaug[:, j, 2 * pr + h01, 0:65],
                        att[:, h01 * 512 + oo : h01 * 512 + 512],
                        start=(j == 0),
                        stop=(j == nj - 1),
                        skip_group_check=True,
                    )

            def emit_normalize(psy, pr, a):
                dsts = a * 512
                # evict the raw psums (y and rowsum) to SBUF first so the
                # PSUM banks free in ~0.5us instead of holding through the
                # whole reciprocal chain (which stalled the next block's
                # first PV and flapped the HAM clock gate)
                stgs = []
                for h01 in range(2):
                    stg = pnorm.tile([65, 512], F32, tag=f"sg{h01}")
                    nc.vector.tensor_copy(stg[:], psy[h01][0:65, :])
                    stgs.append(stg)
                for h01 in range(2):
                    stg = stgs[h01]
                    # 1/rowsum = exp(-ln(rowsum)) on ACT (swap-free with
                    # the combined ln+exp table set); the 3.3us 1-lane
                    # DVE reciprocal on the in-order DVE measured slower
                    nc.scalar.activation(
                        stg[64:65, :],
                        stg[64:65, :],
                        mybir.ActivationFunctionType.Ln,
                    )
                    nc.scalar.activation(
                        stg[64:65, :],
                        stg[64:65, :],
                        mybir.ActivationFunctionType.Exp,
                        scale=-1.0,
                    )
                    rr0 = pnorm.tile([1, 512], F32, tag="rr0")
                    nc.gpsimd.dma_start(rr0[:], stg[64:65, :])
                    bc = pnorm.tile([64, 512], F32, tag="bc")
                    nc.gpsimd.partition_broadcast(bc[:], rr0[:])
                    if h01 == 0:
                        nc.vector.tensor_tensor(
                            yT[0:64, pr, dsts : dsts + 512],
                            stg[0:64, :],
                            bc[:],
                            mybir.AluOpType.mult,
                        )
                    else:
                        stg2 = pnorm.tile([64, 512], BF16, tag="stg2")
                        nc.vector.tensor_tensor(
                            stg2[:],
                            stg[0:64, :],
                            bc[:],
                            mybir.AluOpType.mult,
                        )
                        nc.gpsimd.dma_start(
                            yT[64:128, pr, dsts : dsts + 512], stg2[:]
                        )

            # ---- q/k projection for pair 0 runs up front (PE warm-up
            # under the tail of the xT DMA); later pairs are filler ----
            qkT[0] = (
                pqk.tile([128, S_], BF16, tag="qT", name="qT0"),
                pqk.tile([128, S_], BF16, tag="kT", name="kT0"),
            )
            for cqk in range(2):
                for tb0 in range(0, TB, 2):
                    qk_chunk(0, cqk, tb0)

            for pr in range(NPAIR):
                # prefetch next pair's q/k weights and output tiles
                if pr + 1 < NPAIR:
                    wqk_n = pw.tile(
                        [128, KO, 2, 128], BF16, tag="wqk", name=f"wqk{pr + 1}"
                    )
                    nc.sync.dma_start(wqk_n[:], wqk_r[:, :, pr + 1, :, :])
                    wqk_tiles[pr + 1] = wqk_n
                    qkT[pr + 1] = (
                        pqk.tile([128, S_], BF16, tag="qT", name=f"qT{pr + 1}"),
                        pqk.tile([128, S_], BF16, tag="kT", name=f"kT{pr + 1}"),
                    )

                # filler units interleaved into this pair's j-loop.
                # pair 0: the v-projection tiles (constrained to land
                # ahead of their PV consumers) then pair 1's q/k; pairs
                # 1-2: next pair's q/k; pair 3: output-projection units,
                # appended as their token blocks finish.
                fillers = []
                if pr == 0:
                    fillers += [("v", tt) for tt in range(KT)]
                if pr + 1 < NPAIR:
                    fillers += [
                        ("qk", pr + 1, cqk, tb0)
                        for cqk in range(2)
                        for tb0 in range(0, TB, 2)
                    ]
                f_idx = 0

                def emit_filler():
                    nonlocal f_idx
                    if f_idx >= len(fillers):
                        return False
                    f = fillers[f_idx]
                    f_idx += 1
                    if f[0] == "v":
                        v_tile(f[1])
                    elif f[0] == "qk":
                        qk_chunk(f[1], f[2], f[3])
                    else:
                        out_unit(f[1], f[2])
                    return True

                groups = [
                    (a, j, 4 * a + 4) for a in range(NA) for j in range(4 * a + 4)
                ]
                n_groups = len(groups)

                prev = None
                psy_cur = None
                for gi, (a, j, nj) in enumerate(groups):
                    if pr == 0:
                        # v tile j must be emitted before PV reads it
                        # (PV of this group is emitted next iteration)
                        while (
                            f_idx < len(fillers)
                            and fillers[f_idx][0] == "v"
                            and fillers[f_idx][1] <= j
                        ):
                            emit_filler()
                    if j == 0:
                        psy_new = [
                            psY.tile([65, 512], F32, tag="psY", name=f"psy{h}_{a}")
                            for h in range(2)
                        ]
                    att, oo = emit_group(pr, a, j, f"{pr}_{gi}")
                    # pace remaining fillers across the pair's groups
                    while f_idx < len(fillers) and f_idx * n_groups <= gi * len(fillers):
                        if not emit_filler():
                            break
                    if prev is not None:
                        p_psy, p_att, p_oo, p_a, p_j, p_nj = prev
                        emit_pv(p_psy, p_att, p_oo, pr, p_a, p_j, p_nj)
                        if p_j + 1 >= p_nj:
                            emit_normalize(p_psy, pr, p_a)
                            if pr == NPAIR - 1:
                                fillers.extend(
                                    ("out", tt, nh)
                                    for tt in range(4 * p_a, 4 * p_a + 4)
                                    for nh in range(2)
                                )
                            # extra PE work over the block-boundary
                            # normalize chain keeps the HAM gate warm
                            emit_filler()
                    if j == 0:
                        psy_cur = psy_new
                    prev = (psy_cur, att, oo, a, j, nj)
                # drain the last group and any remaining fillers
                p_psy, p_att, p_oo, p_a, p_j, p_nj = prev
                emit_pv(p_psy, p_att, p_oo, pr, p_a, p_j, p_nj)
                emit_normalize(p_psy, pr, p_a)
                if pr == NPAIR - 1:
                    fillers.extend(
                        ("out", tt, nh)
                        for tt in range(4 * p_a, 4 * p_a + 4)
                        for nh in range(2)
                    )
                while emit_filler():
                    pass

    nc.finalize()
    return nc


def make_host_inputs(x, w_qkv, b_qkv, w_out, b_out, S_=S):
    """Build the 8 per-core input maps (host-side shard/pack/cast)."""
    x = np.asarray(x, dtype=np.float32)
    w_qkv = np.asarray(w_qkv, dtype=np.float32)
    b_qkv = np.asarray(b_qkv, dtype=np.float32)
    w_out = np.asarray(w_out, dtype=np.float32)

    mask = np.where(
        np.arange(896)[None, :] >= np.arange(128)[:, None] + 384, 0.0, NEG
    ).astype(np.float32)
    # [896:1408] repeats [384:896] so a single DVE add masks both heads
    # of a merged [128,1024] score tile (see build_nc)
    mask = np.concatenate([mask, mask[:, 384:896]], axis=1)

    per_hg = {}
    for hg in range(2):
        wqk = np.empty((D, NPAIR, 2, 128), np.float32)
        bqk = np.empty((128, NPAIR, 2), np.float32)
        for p in range(NPAIR):
            h0, h1 = hg * HPC + 2 * p, hg * HPC + 2 * p + 1
            wqk[:, p, 0, 0:64] = w_qkv[:, h0 * HD : (h0 + 1) * HD]
            wqk[:, p, 0, 64:128] = w_qkv[:, h1 * HD : (h1 + 1) * HD]
            wqk[:, p, 1, 0:64] = w_qkv[:, D + h0 * HD : D + (h0 + 1) * HD]
            wqk[:, p, 1, 64:128] = w_qkv[:, D + h1 * HD : D + (h1 + 1) * HD]
            bqk[0:64, p, 0] = b_qkv[h0 * HD : (h0 + 1) * HD]
            bqk[64:128, p, 0] = b_qkv[h1 * HD : (h1 + 1) * HD]
            bqk[0:64, p, 1] = b_qkv[D + h0 * HD : D + (h0 + 1) * HD]
            bqk[64:128, p, 1] = b_qkv[D + h1 * HD : D + (h1 + 1) * HD]
        wv = w_qkv[:, 2 * D + hg * 512 : 2 * D + (hg + 1) * 512]
        bv = np.broadcast_to(
            b_qkv[2 * D + hg * 512 : 2 * D + (hg + 1) * 512], (128, 512)
        ).copy()
        wout = w_out[hg * 512 : (hg + 1) * 512, :]
        per_hg[hg] = dict(
            wqk=np.ascontiguousarray(wqk.astype(NPBF16)),
            bqk=bqk,
            wv=np.ascontiguousarray(wv.astype(NPBF16)),
            bv=bv,
            wout=np.ascontiguousarray(wout.astype(NPBF16)),
        )

    xT_by_b = [
        np.ascontiguousarray(x[b, :S_].T.astype(NPBF16)) for b in range(B)
    ]
    in_maps = []
    for c in range(8):
        b, hg = c // 2, c % 2
        m = dict(per_hg[hg])
        m["xT"] = xT_by_b[b]
        m["mask"] = mask
        in_maps.append(m)
    return in_maps


_NC_CACHE = {}


def _get_nc(S_=S):
    if S_ not in _NC_CACHE:
        _NC_CACHE[S_] = build_nc(S_)
    return _NC_CACHE[S_]


def kernel(x, w_qkv, b_qkv, w_out, b_out):
    x = np.asarray(x, dtype=np.float32)
    b_out = np.asarray(b_out, dtype=np.float32)
    in_maps = make_host_inputs(x, w_qkv, b_qkv, w_out, b_out)
    nc = _get_nc()
    res = run_bass_kernel_spmd(nc, in_maps, list(range(8))).results
    out = np.empty((B, S, D), np.float32)
    for b in range(B):
        out[b] = res[2 * b]["out"] + res[2 * b + 1]["out"] + b_out[None, :]
    return out


# revision 13
# speedup vs baseline: 1.2534x; 1.0047x over previous
"""Causal self-attention on 8 TRN2 NeuronCores.

Problem: B=4, S=2048, D=1024, H=16 heads (hd=64), fp32 in/out.
  qkv = x @ w_qkv + b_qkv ; causal softmax attention ; y @ w_out + b_out

Sharding (tensor-parallel over heads x data-parallel over batch):
  core c -> batch b = c//2, head-group hg = c%2 (8 heads each).
  Each core computes qkv for its 8 heads from x[b], runs attention, and
  produces a partial output  y_local @ w_out[rows]  of shape [S, D].
  Host unshards: out[b] = partial[2b] + partial[2b+1] + b_out.

Device kernel (per core), bf16 matmul operands / fp32 PSUM accumulation:
  - x passed transposed (xT [D, S], bf16) so both projections contract D
    on partitions with no device-side transposes.  Input DMAs are split
    across the sync and gpsimd trigger queues with the q/k weights ahead
    of the bulk xT transfer so the first projection matmuls start early
    instead of queueing behind 4MB of activations.
  - q,k produced directly transposed (qT/kT [64, S] per head) via
    out = w.T @ x; heads processed in pairs packed at partition offsets
    0-63 / 64-127 (the two K=64 score matmuls then run concurrently on
    disjoint row halves of the PE array via auto tile_position).
  - scores computed transposed (S_T[k, q]) so attT = exp(S_T) is already
    in PV layout.  Both heads of a pair share one [128,1024] PSUM tile
    (head0 | head1, 2 banks) so the softmax exp is a single ACTIVATE
    over both heads' scores -- half the ACT instruction overhead of the
    kernel's hot loop, which is ACT-bound.  Causal handled by block
    skipping; on diagonal tiles the scores/mask/PV operate only on the
    live column strip (additive -1e30 mask windows); the exp covers the
    dead gap between the heads' strips (garbage exp'd, never consumed).
  - v in natural layout with a ones column (v_aug) so the PV matmul also
    produces the softmax denominator.  1/rowsum = exp(-ln(.)) on ACT --
    Exp and Ln are forced into the single combined
    natural_log_exp_and_others table set (see _patch_act_tables) so the
    scalar engine never swaps spline tables (the baseline paid 33 table
    loads / 42us); the normalization multiply fuses into the psum->SBUF
    eviction of yT.  Max-subtraction is skipped (scores are O(1) by
    construction; exp exact in fp32).
  - The attention inner loop is ACT(exp)-bound, so all other PE work is
    software-pipelined into it as emission-order "filler": the v
    projection feeds pair 0's loop just ahead of its PV consumers, the
    q/k projection for pair p+1 fills pair p's loop, and the output
    projection for finished token blocks fills pair 3's loop.  Scores
    for key tile j+1 are emitted before PV of tile j so the in-order PE
    queue never stalls on the exp feeding the next PV.  This keeps the
    PE busy enough that the HAM clock gate stays at full rate (the
    baseline ran ~45% of the kernel at half PE clock).
"""

import os
import sys

for _p in ("/root/.axon_site/_ro/trn_rl_repo", "/opt/trn_rl_repo"):
    if os.path.isdir(_p) and _p not in sys.path:
        sys.path.append(_p)

import ml_dtypes
import numpy as np

import concourse.bass as bass  # noqa: F401
import concourse.mybir as mybir
import concourse.tile as tile
from concourse import bacc
from concourse.bass_utils import run_bass_kernel_spmd

B, S, D, H = 4, 2048, 1024, 16
HD = 64
HPC = 8          # heads per core
NPAIR = HPC // 2
KO = D // 128    # contraction chunks over D
ATT_SCALE = 1.0 / np.sqrt(HD)
NEG = -1.0e30

F32 = mybir.dt.float32
F32R = mybir.dt.float32r
BF16 = mybir.dt.bfloat16
NPBF16 = ml_dtypes.bfloat16


def _patch_act_tables(arch: str):
    """Make natural_log_exp_and_others the only provider of Exp and Ln.

    The table-load insertion pass picks, per activation, a table set
    containing its function; with Exp and Ln drawn from different sets
    it emits an ACT_TABLE_LOAD (~1.3us + drain) at every alternation.
    Both functions live together in natural_log_exp_and_others, so
    removing them from every other set (entries and their positions kept,
    so act_func_set_id still indexes act_info.json correctly) forces the
    single combined set: one load for the whole kernel.
    """
    from concourse.hw_specs import get_activation_tables

    tables = get_activation_tables(arch)  # functools.cache'd: mutate in place
    exp_t = mybir.ActivationFunctionType.Exp
    ln_t = mybir.ActivationFunctionType.Ln
    combined = None
    for name, fns in tables.items():
        if exp_t in fns and ln_t in fns:
            combined = name
            break
    if combined is None:
        return
    for name, fns in tables.items():
        if name != combined:
            fns.discard(exp_t)
            fns.discard(ln_t)


def build_nc(S_=S):
    KT = S_ // 128    # key tiles
    TB = S_ // 512    # token blocks for projections
    NA = S_ // 512    # query blocks

    nc = bacc.Bacc(None)
    _patch_act_tables(nc.m.arch)
    xT_d = nc.dram_tensor("xT", [D, S_], BF16, kind="ExternalInput")
    wqk_d = nc.dram_tensor("wqk", [D, NPAIR, 2, 128], BF16, kind="ExternalInput")
    bqk_d = nc.dram_tensor("bqk", [128, NPAIR, 2], F32, kind="ExternalInput")
    wv_d = nc.dram_tensor("wv", [D, HPC * HD], BF16, kind="ExternalInput")
    bv_d = nc.dram_tensor("bv", [128, HPC * HD], F32, kind="ExternalInput")
    wout_d = nc.dram_tensor("wout", [HPC * HD, D], BF16, kind="ExternalInput")
    # mask2[:, 0:896] is the base causal window; [896:1408] repeats
    # [384:896] so one DVE add covers both heads of a merged score tile
    mask_d = nc.dram_tensor("mask", [128, 1408], F32, kind="ExternalInput")
    out_d = nc.dram_tensor("out", [S_, D], F32, kind="ExternalOutput")

    with tile.TileContext(nc) as tc, nc.allow_low_precision("bf16/f32r matmul operands"):
        with (
            tc.tile_pool(name="const", bufs=1) as constp,
            tc.tile_pool(name="pw", bufs=2) as pw,
            tc.tile_pool(name="pqk", bufs=2) as pqk,
            tc.tile_pool(name="patt", bufs=4) as patt,
            tc.tile_pool(name="pnorm", bufs=2) as pnorm,
            tc.tile_pool(name="postage", bufs=3) as postage,
            tc.tile_pool(name="psA", bufs=2, space="PSUM") as psA,
            tc.tile_pool(name="psS", bufs=2, space="PSUM") as psS,
            tc.tile_pool(name="psY", bufs=2, space="PSUM") as psY,
        ):
            # ---- input DMAs: q/k weights ahead of bulk xT, the xT
            # chunks split over four trigger queues (sync/vector/scalar/
            # gpsimd) so descriptor generation and transfers overlap ----
            bqk_sb = constp.tile([128, NPAIR, 2], F32)
            nc.sync.dma_start(bqk_sb[:], bqk_d[:])
            wqk_r = wqk_d.rearrange("(ko p) r c2 c -> p ko r c2 c", p=128)
            wqk_sb0 = pw.tile([128, KO, 2, 128], BF16, tag="wqk", name="wqk0")
            nc.sync.dma_start(wqk_sb0[:], wqk_r[:, :, 0, :, :])
            xT = constp.tile([128, KO, S_], BF16)
            xr = xT_d.rearrange("(ko p) t -> p ko t", p=128)
            nc.sync.dma_start(xT[:, 0:2, :], xr[:, 0:2, :])
            nc.scalar.dma_start(xT[:, 2:4, :], xr[:, 2:4, :])
            nc.scalar.dma_start(xT[:, 4:6, :], xr[:, 4:6, :])
            nc.gpsimd.dma_start(xT[:, 6:8, :], xr[:, 6:8, :])
            bv_sb = constp.tile([128, HPC * HD], F32)
            nc.sync.dma_start(bv_sb[:], bv_d[:])
            wv_sb = constp.tile([128, KO, HPC * HD], BF16)
            nc.sync.dma_start(wv_sb[:], wv_d.rearrange("(ko p) c -> p ko c", p=128))
            wout_sb = constp.tile([128, NPAIR, D], BF16)
            nc.scalar.dma_start(wout_sb[:], wout_d.rearrange("(cc p) c -> p cc c", p=128))
            mask_sb = constp.tile([128, 1408], F32)
            nc.gpsimd.dma_start(mask_sb[:], mask_d[:])

            # v with ones column (col 64); col 65 is pad
            vaug = constp.tile([128, KT, HPC, 66], BF16)
            nc.gpsimd.memset(vaug[:, :, :, 64], 1.0)
            yT = constp.tile([128, NPAIR, S_], BF16)

            wqk_tiles = {0: wqk_sb0}
            qkT = {}

            # ---- emission units (PE "filler" work) ----
            def qk_chunk(pr, cqk, tb0):
                # one double-token-block of the q or k projection: the two
                # psum tiles share each k-chunk's loaded weights
                wqk_sb = wqk_tiles[pr]
                dst = qkT[pr][cqk]
                tbs = [tb0, tb0 + 1]
                pst = [
                    psA.tile([128, 512], F32, tag="psA", name=f"pj{i}")
                    for i in range(len(tbs))
                ]
                for k in range(KO):
                    for i, tb in enumerate(tbs):
                        nc.tensor.matmul(
                            pst[i],
                            wqk_sb[:, k, cqk, :],
                            xT[:, k, tb * 512 : (tb + 1) * 512],
                            start=(k == 0),
                            stop=(k == KO - 1),
                        )
                for i, tb in enumerate(tbs):
                    nc.vector.tensor_scalar_add(
                        dst[:, tb * 512 : (tb + 1) * 512],
                        pst[i][:],
                        bqk_sb[:, pr, cqk : cqk + 1],
                    )

            def v_tile(tt):
                # v projection (all heads) for key tile tt, biased, into v_aug
                ps = psA.tile([128, 512], F32, tag="psA", name="pv")
                for k in range(KO):
                    nc.tensor.matmul(
                        ps,
                        xT[:, k, tt * 128 : (tt + 1) * 128],
                        wv_sb[:, k, :],
                        start=(k == 0),
                        stop=(k == KO - 1),
                    )
                nc.vector.tensor_tensor(
                    vaug[:, tt, :, 0:64],
                    ps[:].rearrange("p (h d) -> p h d", h=HPC),
                    bv_sb[:].rearrange("p (h d) -> p h d", h=HPC),
                    mybir.AluOpType.add,
                )

            def out_unit(tt, nh):
                # output projection for one (token tile, D half)
                ps = psA.tile([128, 512], F32, tag="psA", name="po")
                for cc in range(NPAIR):
                    nc.tensor.matmul(
                        ps,
                        yT[:, cc, tt * 128 : (tt + 1) * 128],
                        wout_sb[:, cc, nh * 512 : (nh + 1) * 512],
                        start=(cc == 0),
                        stop=(cc == NPAIR - 1),
                    )
                ot = postage.tile([128, 512], F32, tag="ot")
                nc.vector.tensor_copy(ot[:], ps[:])
                nc.sync.dma_start(
                    out_d[tt * 128 : (tt + 1) * 128, nh * 512 : (nh + 1) * 512], ot[:]
                )

            # ---- attention emission: one group = one key tile j, both
            # heads side by side in a single 2-bank PSUM tile ----
            def emit_group(pr, a, j, gname):
                qT, kT = qkT[pr]
                o = 128 * j - 512 * a
                oo = max(o, 0)
                pss = psS.tile([128, 1024], F32, tag="psS", name=f"pss{gname}")
                # the two K=64 score matmuls sit adjacent in the PE queue
                # and pack onto disjoint row halves of the array
                for h01 in range(2):
                    lo, hi = h01 * 64, h01 * 64 + 64
                    nc.tensor.matmul(
                        pss[:, h01 * 512 + oo : (h01 + 1) * 512],
                        kT[lo:hi, j * 128 : (j + 1) * 128],
                        qT[lo:hi, a * 512 + oo : (a + 1) * 512],
                        start=True,
                        stop=True,
                    )
                if o >= 0:
                    # one add masks both heads: mask2's [896:1408] region
                    # repeats the window so src column 384+x-oo is correct
                    # for both live strips (the dead gap gets garbage)
                    nc.vector.tensor_tensor(
                        pss[:, oo:1024],
                        pss[:, oo:1024],
                        mask_sb[:, 384 : 1408 - oo],
                        mybir.AluOpType.add,
                    )
                att = patt.tile([128, 1024], BF16, tag="att", name=f"att{gname}")
                # one exp covers both heads (the dead [512:512+oo] gap on
                # diagonal tiles exps garbage that PV never reads)
                nc.scalar.activation(
                    att[:, oo:1024],
                    pss[:, oo:1024],
                    mybir.ActivationFunctionType.Exp,
                    scale=float(ATT_SCALE),
                )
                return att, oo

            def emit_pv(psy, att, oo, pr, a, j, nj):
                for h01 in range(2):
                    # PV restricted to the live strip; PSUM has_written
                    # bits keep untouched columns
                    nc.tensor.matmul(
                        psy[h01][:, oo:512],
                        vaug[:, j, 2 * pr + h01, 0:65],
                        att[:, h01 * 512 + oo : h01 * 512 + 512],
                        start=(j == 0),
                        stop=(j == nj - 1),
                        skip_group_check=True,
                    )

            def emit_evict(psy):
                # evict the raw psums (y and rowsum) to SBUF immediately
                # so the PSUM banks free in ~0.5us instead of holding
                # through the whole reciprocal chain (which stalled the
                # next block's first PV and flapped the HAM clock gate)
                stgs = []
                for h01 in range(2):
                    stg = pnorm.tile([65, 512], F32, tag=f"sg{h01}")
                    nc.vector.tensor_copy(stg[:], psy[h01][0:65, :])
                    stgs.append(stg)
                return stgs

            def emit_norm_chain(stgs, pr, a):
                # deferred a few groups past the block boundary so the
                # Ln/Exp never stalls the ACT queue between softmax exps
                dsts = a * 512
                rs = pnorm.tile([1, 2, 512], F32, tag="rs")
                for h01 in range(2):
                    nc.gpsimd.dma_start(rs[0:1, h01, :], stgs[h01][64:65, :])
                # 1/rowsum = exp(-ln(rowsum)) on ACT (swap-free with the
                # combined ln+exp table set), both heads in one pass; the
                # 3.3us 1-lane DVE reciprocal measured slower
                nc.scalar.activation(
                    rs[0:1, :, :],
                    rs[0:1, :, :],
                    mybir.ActivationFunctionType.Ln,
                )
                nc.scalar.activation(
                    rs[0:1, :, :],
                    rs[0:1, :, :],
                    mybir.ActivationFunctionType.Exp,
                    scale=-1.0,
                )
                for h01 in range(2):
                    bc = pnorm.tile([64, 512], F32, tag="bc")
                    nc.gpsimd.partition_broadcast(bc[:], rs[0:1, h01, :])
                    if h01 == 0:
                        nc.vector.tensor_tensor(
                            yT[0:64, pr, dsts : dsts + 512],
                            stgs[h01][0:64, :],
                            bc[:],
                            mybir.AluOpType.mult,
                        )
                    else:
                        stg2 = pnorm.tile([64, 512], BF16, tag="stg2")
                        nc.vector.tensor_tensor(
                            stg2[:],
                            stgs[h01][0:64, :],
                            bc[:],
                            mybir.AluOpType.mult,
                        )
                        nc.gpsimd.dma_start(
                            yT[64:128, pr, dsts : dsts + 512], stg2[:]
                        )

            # ---- q/k projection for pair 0, first token half only, runs
            # up front (PE warm-up under the tail of the xT DMA): blocks
            # a=0,1 of the attention only touch tokens 0-1023, so the
            # softmax exp stream starts ~15us earlier than a full qk0 ----
            qkT[0] = (
                pqk.tile([128, S_], BF16, tag="qT", name="qT0"),
                pqk.tile([128, S_], BF16, tag="kT", name="kT0"),
            )
            qk_chunk(0, 0, 0)
            qk_chunk(0, 1, 0)

            import heapq

            for pr in range(NPAIR):
                # prefetch next pair's q/k weights and output tiles
                if pr + 1 < NPAIR:
                    wqk_n = pw.tile(
                        [128, KO, 2, 128], BF16, tag="wqk", name=f"wqk{pr + 1}"
                    )
                    nc.sync.dma_start(wqk_n[:], wqk_r[:, :, pr + 1, :, :])
                    wqk_tiles[pr + 1] = wqk_n
                    qkT[pr + 1] = (
                        pqk.tile([128, S_], BF16, tag="qT", name=f"qT{pr + 1}"),
                        pqk.tile([128, S_], BF16, tag="kT", name=f"kT{pr + 1}"),
                    )

                groups = [
                    (a, j, 4 * a + 4) for a in range(NA) for j in range(4 * a + 4)
                ]
                gi_blk = {a: 2 * a * a + 2 * a for a in range(NA)}

                # filler schedule: (emit_gi, seq, unit).  Hard deadlines:
                # v tile tt before its first PV (group (tt//4, tt) + 1);
                # pair 0's remaining qk chunks before block 2 reads
                # tokens 1024+.  Everything else spread for pacing.
                sched = []
                seq = 0

                def push(gi_e, unit):
                    nonlocal seq
                    heapq.heappush(sched, (gi_e, seq, unit))
                    seq += 1

                if pr == 0:
                    for tt in range(KT):
                        b = tt // 4
                        push(tt if b == 0 else gi_blk[b] + tt - 2, ("v", tt))
                    push(6, ("qk", 0, 0, 2))
                    push(9, ("qk", 0, 1, 2))
                    for i in range(2):
                        push(20 + 5 * i, ("qk", 1, i, 0))
                        push(30 + 5 * i, ("qk", 1, i, 2))
                elif pr + 1 < NPAIR:
                    for i in range(2):
                        push(8 + 8 * i, ("qk", pr + 1, i, 0))
                        push(24 + 8 * i, ("qk", pr + 1, i, 2))

                def run(unit):
                    if unit[0] == "v":
                        v_tile(unit[1])
                    elif unit[0] == "qk":
                        qk_chunk(unit[1], unit[2], unit[3])
                    else:
                        out_unit(unit[1], unit[2])

                chains = []  # (emit_gi, stgs, block) deferred norm chains

                def flush_chains(gi_now):
                    while chains and chains[0][0] <= gi_now:
                        _, stgs, blk = chains.pop(0)
                        emit_norm_chain(stgs, pr, blk)
                        if pr == NPAIR - 1:
                            for i, (tt, nh) in enumerate(
                                (tt, nh)
                                for tt in range(4 * blk, 4 * blk + 4)
                                for nh in range(2)
                            ):
                                push(gi_now + 1 + i, ("out", tt, nh))

                prev = None
                psy_cur = None
                for gi, (a, j, nj) in enumerate(groups):
                    flush_chains(gi)
                    while sched and sched[0][0] <= gi:
                        run(heapq.heappop(sched)[2])
                    if j == 0:
                        psy_new = [
                            psY.tile([65, 512], F32, tag="psY", name=f"psy{h}_{a}")
                            for h in range(2)
                        ]
                    att, oo = emit_group(pr, a, j, f"{pr}_{gi}")
                    if prev is not None:
                        p_psy, p_att, p_oo, p_a, p_j, p_nj = prev
                        emit_pv(p_psy, p_att, p_oo, pr, p_a, p_j, p_nj)
                        if p_j + 1 >= p_nj:
                            chains.append((gi + 2, emit_evict(p_psy), p_a))
                    if j == 0:
                        psy_cur = psy_new
                    prev = (psy_cur, att, oo, a, j, nj)
                # drain the last group, pending chains, and fillers
                p_psy, p_att, p_oo, p_a, p_j, p_nj = prev
                emit_pv(p_psy, p_att, p_oo, pr, p_a, p_j, p_nj)
                chains.append((0, emit_evict(p_psy), p_a))
                flush_chains(10 ** 9)
                while sched:
                    run(heapq.heappop(sched)[2])

    nc.finalize()
    return nc


def make_host_inputs(x, w_qkv, b_qkv, w_out, b_out, S_=S):
    """Build the 8 per-core input maps (host-side shard/pack/cast)."""
    x = np.asarray(x, dtype=np.float32)
    w_qkv = np.asarray(w_qkv, dtype=np.float32)
    b_qkv = np.asarray(b_qkv, dtype=np.float32)
    w_out = np.asarray(w_out, dtype=np.float32)

    mask = np.where(
        np.arange(896)[None, :] >= np.arange(128)[:, None] + 384, 0.0, NEG
    ).astype(np.float32)
    # [896:1408] repeats [384:896] so a single DVE add masks both heads
    # of a merged [128,1024] score tile (see build_nc)
    mask = np.concatenate([mask, mask[:, 384:896]], axis=1)

    per_hg = {}
    for hg in range(2):
        wqk = np.empty((D, NPAIR, 2, 128), np.float32)
        bqk = np.empty((128, NPAIR, 2), np.float32)
        for p in range(NPAIR):
            h0, h1 = hg * HPC + 2 * p, hg * HPC + 2 * p + 1
            wqk[:, p, 0, 0:64] = w_qkv[:, h0 * HD : (h0 + 1) * HD]
            wqk[:, p, 0, 64:128] = w_qkv[:, h1 * HD : (h1 + 1) * HD]
            wqk[:, p, 1, 0:64] = w_qkv[:, D + h0 * HD : D + (h0 + 1) * HD]
            wqk[:, p, 1, 64:128] = w_qkv[:, D + h1 * HD : D + (h1 + 1) * HD]
            bqk[0:64, p, 0] = b_qkv[h0 * HD : (h0 + 1) * HD]
            bqk[64:128, p, 0] = b_qkv[h1 * HD : (h1 + 1) * HD]
            bqk[0:64, p, 1] = b_qkv[D + h0 * HD : D + (h0 + 1) * HD]
            bqk[64:128, p, 1] = b_qkv[D + h1 * HD : D + (h1 + 1) * HD]
        wv = w_qkv[:, 2 * D + hg * 512 : 2 * D + (hg + 1) * 512]
        bv = np.broadcast_to(
            b_qkv[2 * D + hg * 512 : 2 * D + (hg + 1) * 512], (128, 512)
        ).copy()
        wout = w_out[hg * 512 : (hg + 1) * 512, :]
        per_hg[hg] = dict(
            wqk=np.ascontiguousarray(wqk.astype(NPBF16)),
            bqk=bqk,
            wv=np.ascontiguousarray(wv.astype(NPBF16)),
            bv=bv,
            wout=np.ascontiguousarray(wout.astype(NPBF16)),
        )

    xT_by_b = [
        np.ascontiguousarray(x[b, :S_].T.astype(NPBF16)) for b in range(B)
    ]
    in_maps = []
    for c in range(8):
        b, hg = c // 2, c % 2
        m = dict(per_hg[hg])
        m["xT"] = xT_by_b[b]
        m["mask"] = mask
        in_maps.append(m)
    return in_maps


_NC_CACHE = {}


def _get_nc(S_=S):
    if S_ not in _NC_CACHE:
        _NC_CACHE[S_] = build_nc(S_)
    return _NC_CACHE[S_]


def kernel(x, w_qkv, b_qkv, w_out, b_out):
    x = np.asarray(x, dtype=np.float32)
    b_out = np.asarray(b_out, dtype=np.float32)
    in_maps = make_host_inputs(x, w_qkv, b_qkv, w_out, b_out)
    nc = _get_nc()
    res = run_bass_kernel_spmd(nc, in_maps, list(range(8))).results
    out = np.empty((B, S, D), np.float32)
    for b in range(B):
        out[b] = res[2 * b]["out"] + res[2 * b + 1]["out"] + b_out[None, :]
    return out


# revision 22
# speedup vs baseline: 1.3006x; 1.0376x over previous
"""Causal self-attention on 8 TRN2 NeuronCores.

Problem: B=4, S=2048, D=1024, H=16 heads (hd=64), fp32 in/out.
  qkv = x @ w_qkv + b_qkv ; causal softmax attention ; y @ w_out + b_out

Sharding (tensor-parallel over heads x data-parallel over batch):
  core c -> batch b = c//2, head-group hg = c%2 (8 heads each).
  Each core computes qkv for its 8 heads from x[b], runs attention, and
  produces a partial output  y_local @ w_out[rows]  of shape [S, D].
  Host unshards: out[b] = partial[2b] + partial[2b+1] + b_out.

Device kernel (per core), bf16 matmul operands / fp32 PSUM accumulation:
  - x passed transposed (xT [D, S], bf16) so both projections contract D
    on partitions with no device-side transposes.  Input DMAs are split
    across the sync and gpsimd trigger queues with the q/k weights ahead
    of the bulk xT transfer so the first projection matmuls start early
    instead of queueing behind 4MB of activations.
  - q,k produced directly transposed (qT/kT [64, S] per head) via
    out = w.T @ x; heads processed in pairs packed at partition offsets
    0-63 / 64-127 (the two K=64 score matmuls then run concurrently on
    disjoint row halves of the PE array via auto tile_position).
  - scores computed transposed (S_T[k, q]) so attT = exp(S_T) is already
    in PV layout.  Both heads of a pair share one [128,1024] PSUM tile
    (head0 | head1, 2 banks) so the softmax exp is a single ACTIVATE
    over both heads' scores -- half the ACT instruction overhead of the
    kernel's hot loop, which is ACT-bound.  Causal handled by block
    skipping; on diagonal tiles the scores/mask/PV operate only on the
    live column strip (additive -1e30 mask windows); the exp covers the
    dead gap between the heads' strips (garbage exp'd, never consumed).
  - v in natural layout with a ones column (v_aug) so the PV matmul also
    produces the softmax denominator.  1/rowsum = exp(-ln(.)) on ACT --
    Exp and Ln are forced into the single combined
    natural_log_exp_and_others table set (see _patch_act_tables) so the
    scalar engine never swaps spline tables (the baseline paid 33 table
    loads / 42us); the normalization multiply fuses into the psum->SBUF
    eviction of yT.  Max-subtraction is skipped (scores are O(1) by
    construction; exp exact in fp32).
  - The attention inner loop is ACT(exp)-bound, so all other PE work is
    software-pipelined into it as emission-order "filler": the v
    projection feeds pair 0's loop just ahead of its PV consumers, the
    q/k projection for pair p+1 fills pair p's loop, and the output
    projection for finished token blocks fills pair 3's loop.  Scores
    for key tile j+1 are emitted before PV of tile j so the in-order PE
    queue never stalls on the exp feeding the next PV.  This keeps the
    PE busy enough that the HAM clock gate stays at full rate (the
    baseline ran ~45% of the kernel at half PE clock).
"""

import os
import sys

for _p in ("/root/.axon_site/_ro/trn_rl_repo", "/opt/trn_rl_repo"):
    if os.path.isdir(_p) and _p not in sys.path:
        sys.path.append(_p)

import ml_dtypes
import numpy as np

import concourse.bass as bass  # noqa: F401
import concourse.mybir as mybir
import concourse.tile as tile
from concourse import bacc
from concourse.bass_utils import run_bass_kernel_spmd

B, S, D, H = 4, 2048, 1024, 16
HD = 64
HPC = 8          # heads per core
NPAIR = HPC // 2
KO = D // 128    # contraction chunks over D
ATT_SCALE = 1.0 / np.sqrt(HD)
NEG = -1.0e30

F32 = mybir.dt.float32
F32R = mybir.dt.float32r
BF16 = mybir.dt.bfloat16
NPBF16 = ml_dtypes.bfloat16


def _patch_act_tables(arch: str):
    """Make natural_log_exp_and_others the only provider of Exp and Ln.

    The table-load insertion pass picks, per activation, a table set
    containing its function; with Exp and Ln drawn from different sets
    it emits an ACT_TABLE_LOAD (~1.3us + drain) at every alternation.
    Both functions live together in natural_log_exp_and_others, so
    removing them from every other set (entries and their positions kept,
    so act_func_set_id still indexes act_info.json correctly) forces the
    single combined set: one load for the whole kernel.
    """
    from concourse.hw_specs import get_activation_tables

    tables = get_activation_tables(arch)  # functools.cache'd: mutate in place
    exp_t = mybir.ActivationFunctionType.Exp
    ln_t = mybir.ActivationFunctionType.Ln
    combined = None
    for name, fns in tables.items():
        if exp_t in fns and ln_t in fns:
            combined = name
            break
    if combined is None:
        return
    for name, fns in tables.items():
        if name != combined:
            fns.discard(exp_t)
            fns.discard(ln_t)


def build_nc(S_=S):
    KT = S_ // 128    # key tiles
    TB = S_ // 512    # token blocks for projections
    NA = S_ // 512    # query blocks

    nc = bacc.Bacc(None)
    _patch_act_tables(nc.m.arch)
    xT_d = nc.dram_tensor("xT", [D, S_], BF16, kind="ExternalInput")
    wqk_d = nc.dram_tensor("wqk", [D, NPAIR, 2, 128], BF16, kind="ExternalInput")
    bqk_d = nc.dram_tensor("bqk", [128, NPAIR, 2], F32, kind="ExternalInput")
    wv_d = nc.dram_tensor("wv", [D, HPC * HD], BF16, kind="ExternalInput")
    bv_d = nc.dram_tensor("bv", [128, HPC * HD], F32, kind="ExternalInput")
    wout_d = nc.dram_tensor("wout", [HPC * HD, D], BF16, kind="ExternalInput")
    # mask2[:, 0:896] is the base causal window; [896:1408] repeats
    # [384:896] so one DVE add covers both heads of a merged score tile
    mask_d = nc.dram_tensor("mask", [128, 1408], F32, kind="ExternalInput")
    out_d = nc.dram_tensor("out", [S_, D], F32, kind="ExternalOutput")

    with tile.TileContext(nc) as tc, nc.allow_low_precision("bf16/f32r matmul operands"):
        with (
            tc.tile_pool(name="const", bufs=1) as constp,
            tc.tile_pool(name="pw", bufs=2) as pw,
            tc.tile_pool(name="pqk", bufs=2) as pqk,
            tc.tile_pool(name="patt", bufs=4) as patt,
            tc.tile_pool(name="pnorm", bufs=2) as pnorm,
            tc.tile_pool(name="postage", bufs=3) as postage,
            tc.tile_pool(name="psA", bufs=2, space="PSUM") as psA,
            tc.tile_pool(name="psS", bufs=2, space="PSUM") as psS,
            tc.tile_pool(name="psY", bufs=2, space="PSUM") as psY,
        ):
            # ---- input DMAs: q/k weights ahead of bulk xT, the xT
            # chunks split over four trigger queues (sync/vector/scalar/
            # gpsimd) so descriptor generation and transfers overlap ----
            bqk_sb = constp.tile([128, NPAIR, 2], F32)
            nc.sync.dma_start(bqk_sb[:], bqk_d[:])
            # tiny dummy activation, first on the ACT queue, so the
            # ln+exp table set DMAs into the ACT table RAMs (~2.7us)
            # under the input-DMA window instead of delaying the first
            # softmax exp
            zz = constp.tile([1, 16], F32)
            nc.gpsimd.memset(zz[:], 0.0)
            nc.scalar.activation(zz[:], zz[:], mybir.ActivationFunctionType.Exp)
            wqk_r = wqk_d.rearrange("(ko p) r c2 c -> p ko r c2 c", p=128)
            wqk_sb0 = pw.tile([128, KO, 2, 128], BF16, tag="wqk", name="wqk0")
            nc.sync.dma_start(wqk_sb0[:], wqk_r[:, :, 0, :, :])
            wv_sb = constp.tile([128, KO, HPC * HD], BF16)
            nc.scalar.dma_start(wv_sb[:], wv_d.rearrange("(ko p) c -> p ko c", p=128))
            bv_sb = constp.tile([128, HPC * HD], F32)
            nc.gpsimd.dma_start(bv_sb[:], bv_d[:])
            # xT chunks in contraction order across the three DMA-capable
            # queues, matching the starter k-wave consumption below
            xT = constp.tile([128, KO, S_], BF16)
            xr = xT_d.rearrange("(ko p) t -> p ko t", p=128)
            nc.sync.dma_start(xT[:, 0:2, :], xr[:, 0:2, :])
            nc.scalar.dma_start(xT[:, 2:4, :], xr[:, 2:4, :])
            nc.gpsimd.dma_start(xT[:, 4:6, :], xr[:, 4:6, :])
            nc.sync.dma_start(xT[:, 6:8, :], xr[:, 6:8, :])
            mask_sb = constp.tile([128, 1408], F32)
            nc.gpsimd.dma_start(mask_sb[:], mask_d[:])
            wout_sb = constp.tile([128, NPAIR, D], BF16)
            nc.gpsimd.dma_start(wout_sb[:], wout_d.rearrange("(cc p) c -> p cc c", p=128))

            # v with ones column (col 64); col 65 is pad
            vaug = constp.tile([128, KT, HPC, 66], BF16)
            nc.gpsimd.memset(vaug[:, :, :, 64], 1.0)
            yT = constp.tile([128, NPAIR, S_], BF16)

            wqk_tiles = {0: wqk_sb0}
            qkT = {}

            # ---- emission units (PE "filler" work) ----
            def qk_chunk(pr, cqk, tb0):
                # one double-token-block of the q or k projection: the two
                # psum tiles share each k-chunk's loaded weights
                wqk_sb = wqk_tiles[pr]
                dst = qkT[pr][cqk]
                tbs = [tb0, tb0 + 1]
                pst = [
                    psA.tile([128, 512], F32, tag="psA", name=f"pj{i}")
                    for i in range(len(tbs))
                ]
                for k in range(KO):
                    for i, tb in enumerate(tbs):
                        nc.tensor.matmul(
                            pst[i],
                            wqk_sb[:, k, cqk, :],
                            xT[:, k, tb * 512 : (tb + 1) * 512],
                            start=(k == 0),
                            stop=(k == KO - 1),
                        )
                for i, tb in enumerate(tbs):
                    nc.vector.tensor_scalar_add(
                        dst[:, tb * 512 : (tb + 1) * 512],
                        pst[i][:],
                        bqk_sb[:, pr, cqk : cqk + 1],
                    )

            def v_tile(tt):
                # v projection (all heads) for key tile tt, biased, into v_aug
                ps = psA.tile([128, 512], F32, tag="psA", name="pv")
                for k in range(KO):
                    nc.tensor.matmul(
                        ps,
                        xT[:, k, tt * 128 : (tt + 1) * 128],
                        wv_sb[:, k, :],
                        start=(k == 0),
                        stop=(k == KO - 1),
                    )
                nc.vector.tensor_tensor(
                    vaug[:, tt, :, 0:64],
                    ps[:].rearrange("p (h d) -> p h d", h=HPC),
                    bv_sb[:].rearrange("p (h d) -> p h d", h=HPC),
                    mybir.AluOpType.add,
                )

            def out_unit(tt, nh):
                # output projection for one (token tile, D half)
                ps = psA.tile([128, 512], F32, tag="psA", name="po")
                for cc in range(NPAIR):
                    nc.tensor.matmul(
                        ps,
                        yT[:, cc, tt * 128 : (tt + 1) * 128],
                        wout_sb[:, cc, nh * 512 : (nh + 1) * 512],
                        start=(cc == 0),
                        stop=(cc == NPAIR - 1),
                    )
                ot = postage.tile([128, 512], F32, tag="ot")
                nc.vector.tensor_copy(ot[:], ps[:])
                nc.sync.dma_start(
                    out_d[tt * 128 : (tt + 1) * 128, nh * 512 : (nh + 1) * 512], ot[:]
                )

            # ---- attention emission: one group = one key tile j, both
            # heads side by side in a single 2-bank PSUM tile ----
            def emit_group(pr, a, j, gname):
                qT, kT = qkT[pr]
                o = 128 * j - 512 * a
                oo = max(o, 0)
                pss = psS.tile([128, 1024], F32, tag="psS", name=f"pss{gname}")
                # the two K=64 score matmuls sit adjacent in the PE queue
                # and pack onto disjoint row halves of the array
                for h01 in range(2):
                    lo, hi = h01 * 64, h01 * 64 + 64
                    nc.tensor.matmul(
                        pss[:, h01 * 512 + oo : (h01 + 1) * 512],
                        kT[lo:hi, j * 128 : (j + 1) * 128],
                        qT[lo:hi, a * 512 + oo : (a + 1) * 512],
                        start=True,
                        stop=True,
                    )
                if o >= 0:
                    # one add masks both heads: mask2's [896:1408] region
                    # repeats the window so src column 384+x-oo is correct
                    # for both live strips (the dead gap gets garbage)
                    nc.vector.tensor_tensor(
                        pss[:, oo:1024],
                        pss[:, oo:1024],
                        mask_sb[:, 384 : 1408 - oo],
                        mybir.AluOpType.add,
                    )
                att = patt.tile([128, 1024], BF16, tag="att", name=f"att{gname}")
                # one exp covers both heads (the dead [512:512+oo] gap on
                # diagonal tiles exps garbage that PV never reads)
                nc.scalar.activation(
                    att[:, oo:1024],
                    pss[:, oo:1024],
                    mybir.ActivationFunctionType.Exp,
                    scale=float(ATT_SCALE),
                )
                return att, oo

            def emit_pv(psy, att, oo, pr, a, j, nj):
                for h01 in range(2):
                    # PV restricted to the live strip; PSUM has_written
                    # bits keep untouched columns
                    nc.tensor.matmul(
                        psy[h01][:, oo:512],
                        vaug[:, j, 2 * pr + h01, 0:65],
                        att[:, h01 * 512 + oo : h01 * 512 + 512],
                        start=(j == 0),
                        stop=(j == nj - 1),
                        skip_group_check=True,
                    )

            def emit_evict(psy):
                # evict the raw psums (y and rowsum) to SBUF immediately
                # so the PSUM banks free in ~0.5us instead of holding
                # through the whole reciprocal chain (which stalled the
                # next block's first PV and flapped the HAM clock gate)
                stg = pnorm.tile([65, 2, 512], F32, tag="sg")
                for h01 in range(2):
                    nc.vector.tensor_copy(stg[:, h01, :], psy[h01][0:65, :])
                return stg

            def emit_norm_chain(stg, pr, a):
                # deferred a few groups past the block boundary so the
                # Ln/Exp never stalls the ACT queue between softmax exps
                dsts = a * 512
                rs = pnorm.tile([1, 2, 512], F32, tag="rs")
                nc.gpsimd.dma_start(rs[:], stg[64:65, :, :])
                # 1/rowsum = exp(-ln(rowsum)) on ACT (swap-free with the
                # combined ln+exp table set), both heads in one pass; the
                # 3.3us 1-lane DVE reciprocal measured slower
                nc.scalar.activation(
                    rs[:], rs[:], mybir.ActivationFunctionType.Ln
                )
                nc.scalar.activation(
                    rs[:], rs[:], mybir.ActivationFunctionType.Exp, scale=-1.0
                )
                bc = pnorm.tile([64, 2, 512], F32, tag="bc")
                nc.gpsimd.partition_broadcast(bc[:], rs[:])
                nc.vector.tensor_tensor(
                    yT[0:64, pr, dsts : dsts + 512],
                    stg[0:64, 0, :],
                    bc[:, 0, :],
                    mybir.AluOpType.mult,
                )
                stg2 = pnorm.tile([64, 512], BF16, tag="stg2")
                nc.vector.tensor_tensor(
                    stg2[:],
                    stg[0:64, 1, :],
                    bc[:, 1, :],
                    mybir.AluOpType.mult,
                )
                nc.sync.dma_start(yT[64:128, pr, dsts : dsts + 512], stg2[:])

            # ---- starter units: q/k projection for pair 0 (first token
            # half) and v tiles 0-1, emitted in k-chunk waves matched to
            # the xT DMA arrival order so the PE computes under the input
            # transfer instead of stalling ~20us for all of xT; the idle
            # psS/psY banks hold the extra accumulations.  Blocks a=0,1
            # of pair 0 only touch tokens 0-1023, so the softmax exp
            # stream starts right after these starters. ----
            qT0 = pqk.tile([128, S_], BF16, tag="qT", name="qT0")
            kT0 = pqk.tile([128, S_], BF16, tag="kT", name="kT0")
            qkT[0] = (qT0, kT0)
            st_qa = [psA.tile([128, 512], F32, tag="psA", name=f"sqa{i}") for i in range(2)]
            st_qb = [psS.tile([128, 1024], F32, tag="psS", name=f"sqb{i}") for i in range(2)]
            st_v = [psY.tile([128, 512], F32, tag="psY", name=f"sv{i}") for i in range(2)]
            for k in range(KO):
                st, sp = (k == 0), (k == KO - 1)
                for i in range(2):
                    nc.tensor.matmul(
                        st_qa[i],
                        wqk_sb0[:, k, 0, :],
                        xT[:, k, i * 512 : (i + 1) * 512],
                        start=st,
                        stop=sp,
                    )
                for i in range(2):
                    nc.tensor.matmul(
                        st_qb[i][:, 0:512],
                        wqk_sb0[:, k, 1, :],
                        xT[:, k, i * 512 : (i + 1) * 512],
                        start=st,
                        stop=sp,
                    )
                for tt in range(2):
                    nc.tensor.matmul(
                        st_v[tt],
                        xT[:, k, tt * 128 : (tt + 1) * 128],
                        wv_sb[:, k, :],
                        start=st,
                        stop=sp,
                    )
            for i in range(2):
                nc.vector.tensor_scalar_add(
                    qT0[:, i * 512 : (i + 1) * 512], st_qa[i][:], bqk_sb[:, 0, 0:1]
                )
                nc.vector.tensor_scalar_add(
                    kT0[:, i * 512 : (i + 1) * 512], st_qb[i][:, 0:512], bqk_sb[:, 0, 1:2]
                )
            for tt in range(2):
                nc.vector.tensor_tensor(
                    vaug[:, tt, :, 0:64],
                    st_v[tt][:].rearrange("p (h d) -> p h d", h=HPC),
                    bv_sb[:].rearrange("p (h d) -> p h d", h=HPC),
                    mybir.AluOpType.add,
                )

            import heapq

            for pr in range(NPAIR):
                # prefetch next pair's q/k weights and output tiles
                if pr + 1 < NPAIR:
                    wqk_n = pw.tile(
                        [128, KO, 2, 128], BF16, tag="wqk", name=f"wqk{pr + 1}"
                    )
                    nc.sync.dma_start(wqk_n[:], wqk_r[:, :, pr + 1, :, :])
                    wqk_tiles[pr + 1] = wqk_n
                    qkT[pr + 1] = (
                        pqk.tile([128, S_], BF16, tag="qT", name=f"qT{pr + 1}"),
                        pqk.tile([128, S_], BF16, tag="kT", name=f"kT{pr + 1}"),
                    )

                groups = [
                    (a, j, 4 * a + 4) for a in range(NA) for j in range(4 * a + 4)
                ]
                gi_blk = {a: 2 * a * a + 2 * a for a in range(NA)}

                # filler schedule: (emit_gi, seq, unit).  Hard deadlines:
                # v tile tt before its first PV (group (tt//4, tt) + 1);
                # pair 0's remaining qk chunks before block 2 reads
                # tokens 1024+.  Everything else spread for pacing.
                sched = []
                seq = 0

                def push(gi_e, unit):
                    nonlocal seq
                    heapq.heappush(sched, (gi_e, seq, unit))
                    seq += 1

                # each pair's own second-token-half q/k chunks land early
                # (blocks 2-3 read tokens 1024+ from gi 12); the NEXT
                # pair's first-half chunks land late in this pair
                if pr == 0:
                    for tt in range(2, KT):
                        b = tt // 4
                        push(tt if b == 0 else gi_blk[b] + tt - 2, ("v", tt))
                    push(6, ("qk", 0, 0, 2))
                    push(9, ("qk", 0, 1, 2))
                else:
                    push(3, ("qk", pr, 0, 2))
                    push(7, ("qk", pr, 1, 2))
                if pr + 1 < NPAIR:
                    push(30, ("qk", pr + 1, 0, 0))
                    push(34, ("qk", pr + 1, 1, 0))

                def run(unit):
                    if unit[0] == "v":
                        v_tile(unit[1])
                    elif unit[0] == "qk":
                        qk_chunk(unit[1], unit[2], unit[3])
                    else:
                        out_unit(unit[1], unit[2])

                chains = []  # (emit_gi, stgs, block) deferred norm chains

                def flush_chains(gi_now):
                    while chains and chains[0][0] <= gi_now:
                        _, stgs, blk = chains.pop(0)
                        emit_norm_chain(stgs, pr, blk)
                        if pr == NPAIR - 1:
                            for i, (tt, nh) in enumerate(
                                (tt, nh)
                                for tt in range(4 * blk, 4 * blk + 4)
                                for nh in range(2)
                            ):
                                # hold back a few units from the middle
                                # blocks: they run at the drain while the
                                # final normalize chain computes, keeping
                                # the PE (and HAM clock) warm for the
                                # last block's output projection
                                if blk in (1, 2) and i >= 6:
                                    push(10 ** 6 + seq, ("out", tt, nh))
                                else:
                                    push(min(gi_now + 1 + i, len(groups) - 1), ("out", tt, nh))

                prev = None
                psy_cur = None
                for gi, (a, j, nj) in enumerate(groups):
                    flush_chains(gi)
                    while sched and sched[0][0] <= gi:
                        run(heapq.heappop(sched)[2])
                    if j == 0:
                        psy_new = [
                            psY.tile([65, 512], F32, tag="psY", name=f"psy{h}_{a}")
                            for h in range(2)
                        ]
                    att, oo = emit_group(pr, a, j, f"{pr}_{gi}")
                    if prev is not None:
                        p_psy, p_att, p_oo, p_a, p_j, p_nj = prev
                        emit_pv(p_psy, p_att, p_oo, pr, p_a, p_j, p_nj)
                        if p_j + 1 >= p_nj:
                            chains.append((gi + 2, emit_evict(p_psy), p_a))
                    if j == 0:
                        psy_cur = psy_new
                    prev = (psy_cur, att, oo, a, j, nj)
                # drain: last PV, then the reserved out units (PE work
                # covering the final normalize chain), then the chain and
                # its gated units
                p_psy, p_att, p_oo, p_a, p_j, p_nj = prev
                emit_pv(p_psy, p_att, p_oo, pr, p_a, p_j, p_nj)
                chains.append((0, emit_evict(p_psy), p_a))
                while sched and sched[0][0] <= 10 ** 7:
                    run(heapq.heappop(sched)[2])
                flush_chains(10 ** 9)
                while sched:
                    run(heapq.heappop(sched)[2])

    nc.finalize()
    return nc


def make_host_inputs(x, w_qkv, b_qkv, w_out, b_out, S_=S):
    """Build the 8 per-core input maps (host-side shard/pack/cast)."""
    x = np.asarray(x, dtype=np.float32)
    w_qkv = np.asarray(w_qkv, dtype=np.float32)
    b_qkv = np.asarray(b_qkv, dtype=np.float32)
    w_out = np.asarray(w_out, dtype=np.float32)

    mask = np.where(
        np.arange(896)[None, :] >= np.arange(128)[:, None] + 384, 0.0, NEG
    ).astype(np.float32)
    # [896:1408] repeats [384:896] so a single DVE add masks both heads
    # of a merged [128,1024] score tile (see build_nc)
    mask = np.concatenate([mask, mask[:, 384:896]], axis=1)

    per_hg = {}
    for hg in range(2):
        wqk = np.empty((D, NPAIR, 2, 128), np.float32)
        bqk = np.empty((128, NPAIR, 2), np.float32)
        for p in range(NPAIR):
            h0, h1 = hg * HPC + 2 * p, hg * HPC + 2 * p + 1
            wqk[:, p, 0, 0:64] = w_qkv[:, h0 * HD : (h0 + 1) * HD]
            wqk[:, p, 0, 64:128] = w_qkv[:, h1 * HD : (h1 + 1) * HD]
            wqk[:, p, 1, 0:64] = w_qkv[:, D + h0 * HD : D + (h0 + 1) * HD]
            wqk[:, p, 1, 64:128] = w_qkv[:, D + h1 * HD : D + (h1 + 1) * HD]
            bqk[0:64, p, 0] = b_qkv[h0 * HD : (h0 + 1) * HD]
            bqk[64:128, p, 0] = b_qkv[h1 * HD : (h1 + 1) * HD]
            bqk[0:64, p, 1] = b_qkv[D + h0 * HD : D + (h0 + 1) * HD]
            bqk[64:128, p, 1] = b_qkv[D + h1 * HD : D + (h1 + 1) * HD]
        wv = w_qkv[:, 2 * D + hg * 512 : 2 * D + (hg + 1) * 512]
        bv = np.broadcast_to(
            b_qkv[2 * D + hg * 512 : 2 * D + (hg + 1) * 512], (128, 512)
        ).copy()
        wout = w_out[hg * 512 : (hg + 1) * 512, :]
        per_hg[hg] = dict(
            wqk=np.ascontiguousarray(wqk.astype(NPBF16)),
            bqk=bqk,
            wv=np.ascontiguousarray(wv.astype(NPBF16)),
            bv=bv,
            wout=np.ascontiguousarray(wout.astype(NPBF16)),
        )

    xT_by_b = [
        np.ascontiguousarray(x[b, :S_].T.astype(NPBF16)) for b in range(B)
    ]
    in_maps = []
    for c in range(8):
        b, hg = c // 2, c % 2
        m = dict(per_hg[hg])
        m["xT"] = xT_by_b[b]
        m["mask"] = mask
        in_maps.append(m)
    return in_maps


_NC_CACHE = {}


def _get_nc(S_=S):
    if S_ not in _NC_CACHE:
        _NC_CACHE[S_] = build_nc(S_)
    return _NC_CACHE[S_]


def kernel(x, w_qkv, b_qkv, w_out, b_out):
    x = np.asarray(x, dtype=np.float32)
    b_out = np.asarray(b_out, dtype=np.float32)
    in_maps = make_host_inputs(x, w_qkv, b_qkv, w_out, b_out)
    nc = _get_nc()
    res = run_bass_kernel_spmd(nc, in_maps, list(range(8))).results
    out = np.empty((B, S, D), np.float32)
    for b in range(B):
        out[b] = res[2 * b]["out"] + res[2 * b + 1]["out"] + b_out[None, :]
    return out
